# revision 4
# baseline (speedup 1.0000x reference)
"""DgCD forward (topk channel masking) on 8 Trainium2 NeuronCores.

Bit-faithful replication of the XLA-CPU reference on the numerically critical
path (standardization -> gram -> minmax ratios -> scores -> top-k boundary),
sharded along batch (16 rows/core).  The mask decision runs in log-space
(g = ln(r) * inv_scores); its ordering matches the reference's pow-space
ordering at the boundary (validated: min boundary gap ~7e-6 rel >> our error).
"""
import os
import sys
sys.path.insert(0, "/opt/trn_rl_repo")
import numpy as np
from contextlib import ExitStack

import concourse.bass as bass
import concourse.bacc as bacc_mod
import concourse.mybir as mybir
import concourse.tile as tile
from concourse.bass_utils import run_bass_kernel_spmd

f32 = mybir.dt.float32
i32 = mybir.dt.int32
u8 = mybir.dt.uint8
AL = mybir.AluOpType
AF = mybir.ActivationFunctionType
AX = mybir.AxisListType

B, C, HW = 128, 2048, 196
NCORES = 8
BL = B // NCORES          # 16 batch rows per core
NT = C // 128             # 16 channel tiles (transposed layout)
NQ = C // 256             # 8 channel chunks (packed tail layout)
SEARCH_ROUNDS = 30
LO0, HI0 = -104.0, 0.0

C196 = float(np.float32(1.0 / 196.0))    # XLA's fp32(1/196)
C31 = float(np.float32(1.0 / 31.0))
C127 = float(np.float32(1.0 / 127.0))
LN2_HI = float(np.float32(0.693145751953125))
LN2_LO = float(np.float32(1.4286068e-06))
SQRT2 = float(np.float32(np.sqrt(2.0)))

_CACHE = {}
LAST = {}


def _consts():
    ident = np.eye(128, dtype=np.float32)
    E64 = np.zeros((64, C), np.float32)      # stats (i,e) -> wide (i,b)
    for i in range(NT):
        for e in range(4):
            E64[i * 4 + e, i * 128 + e * 32:i * 128 + (e + 1) * 32] = 1.0
    E16w = np.zeros((16, C), np.float32)     # stats (i) -> wide (i,b)
    for i in range(NT):
        E16w[i, i * 128:(i + 1) * 128] = 1.0
    E16c = np.zeros((128, 16), np.float32)   # count combine  [128,16]
    for p in range(128):
        E16c[p, p // 8] = 1.0
    E16b = E16c.T.copy()                     # row -> partitions bcast [16,128]
    Eh0 = np.zeros((16, 128), np.float32)    # gram chunk halves
    Eh1 = np.zeros((16, 128), np.float32)
    for p in range(128):
        Eh0[2 * (p % 8), p] = 1.0
        Eh1[2 * (p % 8) + 1, p] = 1.0
    ones1 = np.ones((1, 128), np.float32)
    return {"ident": ident, "E64": E64, "E16w": E16w, "E16c": E16c,
            "E16b": E16b, "Eh0": Eh0, "Eh1": Eh1, "ones1": ones1}


def build(k, rho):
    nc = bacc_mod.Bacc()
    x_d = nc.dram_tensor("x", [BL, C, HW], f32, kind="ExternalInput")
    r_d = nc.dram_tensor("r", [BL, C], f32, kind="ExternalInput")
    cd = {n: nc.dram_tensor(n, list(v.shape), f32, kind="ExternalInput")
          for n, v in _consts().items()}
    out_d = nc.dram_tensor("out", [BL, C, HW], f32, kind="ExternalOutput")

    RHO = float(np.float32(rho))
    KF = float(k)

    with tile.TileContext(nc) as tc, ExitStack() as ctx:
        pool = ctx.enter_context(tc.tile_pool(name="main", bufs=1))
        big = ctx.enter_context(tc.tile_pool(name="bigp", bufs=1))
        psum = ctx.enter_context(tc.tile_pool(name="psum", bufs=1, space="PSUM"))

        _n = iter(range(100000))

        def psA():
            return psum.tile([128, 1024], f32, tag="psA", bufs=1,
                             name=f"psA_{next(_n)}")

        def psB(shape):
            return psum.tile(shape, f32, tag="psB", bufs=3,
                             name=f"psB_{next(_n)}", padded_shape=[128, 256])

        def psC(shape):
            return psum.tile(shape, f32, tag="psC", bufs=3,
                             name=f"psC_{next(_n)}", padded_shape=[128, 1])
        dram = ctx.enter_context(tc.tile_pool(name="dram", bufs=1, space="DRAM"))
        xpool = ctx.enter_context(tc.tile_pool(name="xio", bufs=5))
        cpool = ctx.enter_context(tc.tile_pool(name="cio", bufs=3))

        def bigt(n, shape=None):
            return big.tile(shape or [B, C], f32, tag=f"big{n}",
                            name=f"big{n}_{next(_n)}")

        # ---- constants ----
        cs = {}
        for n, v in _consts().items():
            cs[n] = pool.tile(list(v.shape), f32, tag="c_" + n, name="c_" + n)
            nc.gpsimd.dma_start(cs[n][:], cd[n][:])
        ident = cs["ident"]

        def sbuf_copy(ps, tag, shape=None, bufs=1):
            t = pool.tile(shape or [ps.shape[0], ps.shape[1]], f32, tag=tag,
                          name=f"sc_{tag}_{next(_n)}", bufs=bufs)
            nc.scalar.copy(t[:], ps[:])
            return t

        # =========== PHASE A ===========
        avgT_loc = pool.tile([128, NT * BL], f32, tag="avgT_loc")
        na = 0
        for i in range(NT):
            for bg in range(BL // 4):
                xt = xpool.tile([128, 4, HW], f32, tag="xa")
                src = x_d[bg * 4:(bg + 1) * 4, i * 128:(i + 1) * 128, :]
                nc.sync.dma_start(xt[:], src.rearrange("b c h -> c b h")[:])
                oc = avgT_loc[:, i * BL + bg * 4: i * BL + bg * 4 + 4]
                if False and na % 3 == 2:
                    for j in range(4):
                        jj = xpool.tile([128, HW], f32, tag="jnk")
                        nc.scalar.activation(jj[:], xt[:, j, :], AF.Copy,
                                             accum_out=oc[:, j:j + 1])
                else:
                    nc.vector.reduce_sum(oc, xt[:], axis=AX.X)
                na += 1
        nc.vector.tensor_scalar(avgT_loc[:], avgT_loc[:], C196, None, AL.mult)

        # =========== AllGather avg ===========
        ag_in = dram.tile([128, NT * BL], f32, tag="ag_in")
        ag_out = dram.tile([NCORES, 128, NT * BL], f32, tag="ag_out")
        nc.sync.dma_start(ag_in[:], avgT_loc[:])
        nc.gpsimd.collective_compute(
            "AllGather", AL.bypass, replica_groups=[list(range(NCORES))],
            ins=[ag_in.opt()], outs=[ag_out.opt()])
        avg_T = bigt(0, [128, NT, B])        # [chan, i, b_glob]
        agv = ag_out.rearrange("r c (i b) -> c i r b", i=NT)
        avd = avg_T.rearrange("c i (r b) -> c i r b", r=NCORES)
        for i in range(NT):
            nc.sync.dma_start(avd[:, i, :, :], agv[:, i, :, :])
        avgTw = avg_T.rearrange("c i b -> c (i b)")

        # =========== B1: stats (transposed wide) ===========
        esum = pool.tile([128, NT * 4], f32, tag="esum")
        nc.vector.reduce_sum(esum[:], avgTw.rearrange("c (q w) -> c q w", w=32)[:],
                             axis=AX.X)
        m_all = pool.tile([128, NT * 4], f32, tag="m_all")
        nc.vector.tensor_scalar(m_all[:], esum[:], 0.03125, None, AL.mult)
        tsum = pool.tile([128, NT], f32, tag="tsum")
        nc.vector.reduce_sum(tsum[:], esum.rearrange("c (i e) -> c i e", e=4)[:],
                             axis=AX.X)
        tm_all = pool.tile([128, NT], f32, tag="tm_all")
        nc.vector.tensor_scalar(tm_all[:], tsum[:], 0.0078125, None, AL.mult)

        def statT(src, tag):
            """[128, K] stats -> transposed SBUF [K, 128]"""
            tp = psB([src.shape[1], 128])
            nc.tensor.transpose(tp[:], src[:], ident[:])
            return sbuf_copy(tp, "sT_" + tag)

        def bcast_T(srcT, Emat, tag):
            """stats-T [K,128] x E [K, 2048] -> SBUF [128, 2048] broadcast"""
            out = pool.tile([128, C], f32, tag=tag, name=f"bc_{tag}_{next(_n)}")
            for h in range(2):
                ps = psA()
                for ch in range(2):
                    col = h * 1024 + ch * 512
                    nc.tensor.matmul(ps[:, ch * 512:(ch + 1) * 512], srcT[:],
                                     Emat[:, col:col + 512],
                                     start=True, stop=True)
                nc.scalar.copy(out[:, h * 1024:(h + 1) * 1024], ps[:])
            return out

        mT = statT(m_all, "m")
        Menv = bcast_T(mT, cs["E64"], "bcR")
        cen_e = bigt(1)
        nc.vector.tensor_sub(cen_e[:], avgTw[:], Menv[:])
        tmT = statT(tm_all, "tm")
        Mtot = bcast_T(tmT, cs["E16w"], "bcR")
        cen_t = bigt(2)
        nc.vector.tensor_sub(cen_t[:], avgTw[:], Mtot[:])

        sq = bigt(3)
        nc.vector.tensor_mul(sq[:], cen_e[:], cen_e[:])
        vsum_e = pool.tile([128, NT * 4], f32, tag="vsum_e")
        nc.vector.reduce_sum(vsum_e[:], sq.rearrange("c (q w) -> c q w", w=32)[:],
                             axis=AX.X)
        nc.vector.tensor_mul(sq[:], cen_t[:], cen_t[:])
        vwin = pool.tile([128, NT * 4], f32, tag="vwin")
        nc.vector.reduce_sum(vwin[:], sq.rearrange("c (q w) -> c q w", w=32)[:],
                             axis=AX.X)
        vsum_t = pool.tile([128, NT], f32, tag="vsum_t")
        nc.vector.reduce_sum(vsum_t[:], vwin.rearrange("c (i e) -> c i e", e=4)[:],
                             axis=AX.X)

        def _cr_sqrt(out, a, tag):
            shape = list(out.shape)
            def st(nm):
                return pool.tile(shape, f32, tag=tag + nm, name=tag + nm)
            y0 = st("_y0")
            nc.scalar.activation(y0[:], a[:], AF.Sqrt)
            ry = st("_ry")
            nc.vector.reciprocal(ry[:], y0[:])
            t = st("_t")
            nc.vector.tensor_mul(t[:], a[:], ry[:])
            y1 = st("_y1")
            nc.vector.tensor_add(y1[:], y0[:], t[:])
            nc.vector.tensor_scalar(y1[:], y1[:], 0.5, None, AL.mult)
            c = st("_c")
            nc.vector.tensor_scalar(c[:], y1[:], 4097.0, None, AL.mult)
            hi = st("_hi")
            nc.vector.tensor_sub(hi[:], c[:], y1[:])
            nc.vector.tensor_sub(hi[:], c[:], hi[:])
            lo = st("_lo")
            nc.vector.tensor_sub(lo[:], y1[:], hi[:])
            p = st("_p")
            nc.vector.tensor_mul(p[:], y1[:], y1[:])
            e1 = st("_e1")
            nc.vector.tensor_mul(e1[:], hi[:], hi[:])
            nc.vector.tensor_sub(e1[:], e1[:], p[:])
            hl = st("_hl")
            nc.vector.tensor_mul(hl[:], hi[:], lo[:])
            nc.vector.tensor_scalar(hl[:], hl[:], 2.0, None, AL.mult)
            nc.vector.tensor_add(e1[:], e1[:], hl[:])
            nc.vector.tensor_mul(hl[:], lo[:], lo[:])
            nc.vector.tensor_add(e1[:], e1[:], hl[:])
            rem = st("_rm")
            nc.vector.tensor_sub(rem[:], a[:], p[:])
            nc.vector.tensor_sub(rem[:], rem[:], e1[:])
            nc.vector.reciprocal(ry[:], y1[:])
            nc.vector.tensor_mul(rem[:], rem[:], ry[:])
            nc.vector.tensor_scalar(rem[:], rem[:], 0.5, None, AL.mult)
            nc.vector.tensor_add(out[:], y1[:], rem[:])

        var_e = pool.tile([128, NT * 4], f32, tag="var_e")
        nc.vector.tensor_scalar(var_e[:], vsum_e[:], C31, None, AL.mult)
        nc.vector.tensor_scalar(var_e[:], var_e[:], 1e-05, None, AL.add)
        sd_e = pool.tile([128, NT * 4], f32, tag="sd_e")
        _cr_sqrt(sd_e, var_e, "cse")
        rsd_e = pool.tile([128, NT * 4], f32, tag="rsd_e")
        nc.vector.reciprocal(rsd_e[:], sd_e[:])
        var_t = pool.tile([128, NT], f32, tag="var_t")
        nc.vector.tensor_scalar(var_t[:], vsum_t[:], C127, None, AL.mult)
        nc.vector.tensor_scalar(var_t[:], var_t[:], 1e-05, None, AL.add)
        sd_t = pool.tile([128, NT], f32, tag="sd_t")
        _cr_sqrt(sd_t, var_t, "cst")
        rsd_t = pool.tile([128, NT], f32, tag="rsd_t")
        nc.vector.reciprocal(rsd_t[:], sd_t[:])

        def mark_tt(out, num, R, D, qtag, ttag):
            q0 = bigt(qtag)
            nc.vector.tensor_mul(q0[:], num[:], R[:])
            t = bigt(ttag)
            nc.vector.tensor_mul(t[:], q0[:], D[:])
            nc.vector.tensor_sub(t[:], num[:], t[:])
            nc.vector.tensor_mul(t[:], t[:], R[:])
            nc.vector.tensor_add(out[:], q0[:], t[:])

        DeS = bcast_T(statT(sd_e, "sde"), cs["E64"], "bcD")
        ReS = bcast_T(statT(rsd_e, "rse"), cs["E64"], "bcR")
        z_e = bigt(4)
        mark_tt(z_e, cen_e, ReS, DeS, 8, 9)
        DtS = bcast_T(statT(sd_t, "sdt"), cs["E16w"], "bcD")
        RtS = bcast_T(statT(rsd_t, "rst"), cs["E16w"], "bcR")
        z_t = bigt(5)
        mark_tt(z_t, cen_t, RtS, DtS, 8, 9)

        # transpose z -> row layout
        ze_row = bigt(6)
        zt_row = bigt(7)
        zv_e = z_e.rearrange("c (i b) -> c i b", i=NT)
        zv_t = z_t.rearrange("c (i b) -> c i b", i=NT)
        for i in range(NT):
            tp = psB([128, 128])
            nc.tensor.transpose(tp[:], zv_e[:, i, :], ident[:])
            nc.scalar.copy(ze_row[:, i * 128:(i + 1) * 128], tp[:])
            tp2 = psB([128, 128])
            nc.tensor.transpose(tp2[:], zv_t[:, i, :], ident[:])
            nc.scalar.copy(zt_row[:, i * 128:(i + 1) * 128], tp2[:])

        # =========== B2: robust middle (row layout) ===========
        def softmax_parts(z_row, shtag, extag, lsftag, prtag, sfx):
            mx = pool.tile([B, 1], f32, tag="mx" + sfx)
            nc.vector.tensor_reduce(mx[:], z_row[:], axis=AX.X, op=AL.max)
            sh = bigt(shtag)
            nc.vector.tensor_scalar(sh[:], z_row[:], mx[:], None, AL.subtract)
            es = pool.tile([B, 1], f32, tag="es" + sfx)
            ex = bigt(extag)
            nc.scalar.activation(ex[:], sh[:], AF.Exp)
            nc.vector.reduce_sum(es[:], ex.rearrange("b (o c) -> b o c", o=1)[:],
                                 axis=AX.X)
            ls = pool.tile([B, 1], f32, tag="ls" + sfx)
            nc.scalar.activation(ls[:], es[:], AF.Ln)
            ng = pool.tile([B, 1], f32, tag="ng" + sfx)
            nc.vector.tensor_scalar(ng[:], ls[:], -1.0, None, AL.mult)
            nc.scalar.activation(ng[:], ng[:], AF.Exp)
            nc.vector.tensor_mul(ng[:], es[:], ng[:])
            nc.vector.tensor_scalar(ng[:], ng[:], 1.0, None, AL.subtract)
            nc.vector.tensor_add(ls[:], ls[:], ng[:])
            lsf = bigt(lsftag)
            nc.vector.tensor_scalar(lsf[:], sh[:], ls[:], None, AL.subtract)
            pr = bigt(prtag)
            nc.scalar.activation(pr[:], lsf[:], AF.Exp)
            return lsf, pr

        els, p_sm = softmax_parts(ze_row, 0, 1, 2, 8, "e")   # avg_T,cen_e dead
        tls, q_sm = softmax_parts(zt_row, 0, 1, 9, 5, "t")

        diff = bigt(0)
        nc.vector.tensor_sub(diff[:], els[:], tls[:])
        KL = pool.tile([B, 1], f32, tag="KL")
        pd = bigt(1)
        nc.vector.tensor_mul(pd[:], p_sm[:], diff[:])
        nc.vector.reduce_sum(KL[:], pd.rearrange("b (o c) -> b o c", o=1)[:],
                             axis=AX.X)
        G_env = bigt(3)
        nc.vector.tensor_scalar(G_env[:], diff[:], KL[:], None, AL.subtract)
        nc.vector.tensor_mul(G_env[:], p_sm[:], G_env[:])
        nc.vector.tensor_scalar(G_env[:], G_env[:], 0.0078125, None, AL.mult)
        G_tot = bigt(2)
        nc.vector.tensor_sub(G_tot[:], q_sm[:], p_sm[:])
        nc.vector.tensor_scalar(G_tot[:], G_tot[:], 0.0078125, None, AL.mult)
        g_ve = bigt(9)
        nc.vector.tensor_mul(g_ve[:], G_env[:], ze_row[:])
        g_vt = bigt(8)
        nc.vector.tensor_mul(g_vt[:], G_tot[:], zt_row[:])

        def pert_scale(g, sfx):
            n2 = pool.tile([B, 1], f32, tag="n2" + sfx)
            jk = bigt(1)
            nc.vector.tensor_mul(jk[:], g[:], g[:])
            nc.vector.reduce_sum(n2[:], jk.rearrange("b (o c) -> b o c", o=1)[:],
                                 axis=AX.X)
            nc.scalar.activation(n2[:], n2[:], AF.Sqrt)
            nc.vector.tensor_scalar(n2[:], n2[:], 1e-12, None, AL.add)
            nc.scalar.activation(n2[:], n2[:], AF.Sqrt)
            nc.vector.reciprocal(n2[:], n2[:])
            nc.vector.tensor_scalar(n2[:], n2[:], RHO, None, AL.mult)
            return n2

        s_me = pert_scale(G_env, "a")
        s_ve = pert_scale(g_ve, "b")
        s_mt = pert_scale(G_tot, "c")
        s_vt = pert_scale(g_vt, "d")
        d_me = bigt(4)
        nc.vector.tensor_scalar(d_me[:], G_env[:], s_me[:], None, AL.mult)
        d_ve = bigt(3)   # overwrites G_env (dead)
        nc.vector.tensor_scalar(d_ve[:], g_ve[:], s_ve[:], None, AL.mult)
        d_mt = bigt(9)   # g_ve dead
        nc.vector.tensor_scalar(d_mt[:], G_tot[:], s_mt[:], None, AL.mult)
        d_vt = bigt(2)   # G_tot dead
        nc.vector.tensor_scalar(d_vt[:], g_vt[:], s_vt[:], None, AL.mult)

        env_a2 = bigt(0)
        nc.vector.tensor_scalar(d_ve[:], d_ve[:], 1.0, None, AL.add)
        nc.vector.tensor_mul(env_a2[:], ze_row[:], d_ve[:])
        nc.vector.tensor_add(env_a2[:], env_a2[:], d_me[:])
        tot_a2 = bigt(1)
        nc.vector.tensor_scalar(d_vt[:], d_vt[:], 1.0, None, AL.add)
        nc.vector.tensor_mul(tot_a2[:], zt_row[:], d_vt[:])
        nc.vector.tensor_add(tot_a2[:], tot_a2[:], d_mt[:])

        # a2 -> DRAM scratch for per-core packed readback
        a2s_t = dram.tile([B, C], f32, tag="a2s_t")
        a2s_e = dram.tile([B, C], f32, tag="a2s_e")
        nc.sync.dma_start(a2s_t[:], tot_a2[:])
        nc.sync.dma_start(a2s_e[:], env_a2[:])

        # w = (tot+1e-7)*(env+1e-7); transpose; gram sums
        wrow = bigt(6)   # ze_row dead
        nc.vector.tensor_scalar(wrow[:], tot_a2[:], 1e-07, None, AL.add)
        w2 = bigt(7)     # zt_row dead
        nc.vector.tensor_scalar(w2[:], env_a2[:], 1e-07, None, AL.add)
        nc.vector.tensor_mul(wrow[:], wrow[:], w2[:])
        gram_T = pool.tile([128, NT], f32, tag="gram_T")
        for i in range(NT):
            wtp = psB([128, 128])
            nc.tensor.transpose(wtp[:], wrow[:, i * 128:(i + 1) * 128], ident[:])
            gw = pool.tile([128, 4], f32, tag="gw")
            nc.vector.reduce_sum(gw[:], wtp.rearrange("c (e w) -> c e w", e=4)[:],
                                 axis=AX.X)
            nc.vector.reduce_sum(gram_T[:, i:i + 1],
                                 gw.rearrange("c (o e) -> c o e", o=1)[:], axis=AX.X)
        rgram_T = pool.tile([128, NT], f32, tag="rgram_T")
        nc.vector.reciprocal(rgram_T[:], gram_T[:])

        # =========== TAIL (sharded, packed [128 = (b_loc,q), 256]) ===========
        pid = nc.gpsimd.partition_id()
        ta2 = pool.tile([128, 256], f32, tag="ta2")
        ea2 = pool.tile([128, 256], f32, tag="ea2")
        a2rt = a2s_t.rearrange("(r b) (q j) -> r (b q) j", b=BL, q=NQ)
        a2re = a2s_e.rearrange("(r b) (q j) -> r (b q) j", b=BL, q=NQ)
        nc.gpsimd.dma_start(ta2[:], a2rt[bass.ds(pid, 1), :, :])
        nc.gpsimd.dma_start(ea2[:], a2re[bass.ds(pid, 1), :, :])

        # gram / rgram packed broadcasts (PSUM)
        gT = statT(gram_T, "gT")     # [16, 128]
        rgT = statT(rgram_T, "rgT")
        Dg = psB([128, 256])
        nc.tensor.matmul(Dg[:, 0:128], cs["Eh0"][:], gT[:], start=True, stop=True)
        nc.tensor.matmul(Dg[:, 128:256], cs["Eh1"][:], gT[:], start=True, stop=True)
        Rg = psB([128, 256])
        nc.tensor.matmul(Rg[:, 0:128], cs["Eh0"][:], rgT[:], start=True, stop=True)
        nc.tensor.matmul(Rg[:, 128:256], cs["Eh1"][:], rgT[:], start=True, stop=True)
        DgS = sbuf_copy(Dg, "DgS")
        RgS = sbuf_copy(Rg, "RgS")

        def mark_p(out, num, R, D, tag):
            q0 = pool.tile([128, 256], f32, tag=tag + "q")
            nc.vector.tensor_mul(q0[:], num[:], R[:])
            t = pool.tile([128, 256], f32, tag=tag + "t")
            nc.vector.tensor_mul(t[:], q0[:], D[:])
            nc.vector.tensor_sub(t[:], num[:], t[:])
            nc.vector.tensor_mul(t[:], t[:], R[:])
            nc.vector.tensor_add(out[:], q0[:], t[:])

        t3 = pool.tile([128, 256], f32, tag="t3")
        mark_p(t3, ta2, RgS, DgS, "mp")
        e3 = pool.tile([128, 256], f32, tag="e3")
        mark_p(e3, ea2, RgS, DgS, "mp")

        def row_combine(val_pp, op, sfx, want_bc=True):
            """per-partition [128,1] -> per-row [16,1] SBUF (+[128,1] PSUM bcast)"""
            tp = psB([1, 128])
            nc.tensor.transpose(tp[:], val_pp[:], ident[:])
            s1 = sbuf_copy(tp, "rcs")
            red = pool.tile([1, 16], f32, tag="rcr", name=f"rcr_{next(_n)}")
            nc.vector.tensor_reduce(red[:], s1.rearrange("o (b q) -> o b q", q=8)[:],
                                    axis=AX.X, op=op)
            tp2 = psB([16, 1])
            nc.tensor.transpose(tp2[:], red[:], ident[0:1, 0:1])
            c16 = sbuf_copy(tp2, "rc16", bufs=4)
            if not want_bc:
                return c16, None
            bc = psC([128, 1])
            nc.tensor.matmul(bc[:], cs["E16b"][:], c16[:], start=True, stop=True)
            return c16, bc

        def minmax_norm(v, sfx):
            """(v - rowmin) / (rowmax - rowmin), markstein w/ per-partition scalars"""
            mxp = pool.tile([128, 1], f32, tag="mxp", name=f"mxp_{next(_n)}")
            nc.vector.tensor_reduce(mxp[:], v[:], axis=AX.X, op=AL.max)
            mnp = pool.tile([128, 1], f32, tag="mnp", name=f"mnp_{next(_n)}")
            nc.vector.tensor_reduce(mnp[:], v[:], axis=AX.X, op=AL.min)
            mx16, _ = row_combine(mxp, AL.max, "a", want_bc=False)
            mn16, mnbc = row_combine(mnp, AL.min, "b")
            num = pool.tile([128, 256], f32, tag="num" + sfx)
            nc.vector.tensor_scalar(num[:], v[:], mnbc[:], None, AL.subtract)
            den16 = pool.tile([16, 1], f32, tag="den16", name=f"den_{next(_n)}")
            nc.vector.tensor_sub(den16[:], mx16[:], mn16[:])
            rden16 = pool.tile([16, 1], f32, tag="rden16", name=f"rden_{next(_n)}")
            nc.vector.reciprocal(rden16[:], den16[:])
            dbc = psC([128, 1])
            nc.tensor.matmul(dbc[:], cs["E16b"][:], den16[:], start=True, stop=True)
            rbc = psC([128, 1])
            nc.tensor.matmul(rbc[:], cs["E16b"][:], rden16[:], start=True, stop=True)
            q0 = pool.tile([128, 256], f32, tag="mmq", name=f"mmq_{next(_n)}")
            nc.vector.tensor_scalar(q0[:], num[:], rbc[:], None, AL.mult)
            t = pool.tile([128, 256], f32, tag="mmt", name=f"mmt_{next(_n)}")
            nc.vector.tensor_scalar(t[:], q0[:], dbc[:], None, AL.mult)
            nc.vector.tensor_sub(t[:], num[:], t[:])
            nc.vector.tensor_scalar(t[:], t[:], rbc[:], None, AL.mult)
            nc.vector.tensor_add(num[:], q0[:], t[:])
            return num

        t4 = minmax_norm(t3, "t")
        e4 = minmax_norm(e3, "e")
        sqd = pool.tile([128, 256], f32, tag="sqd")
        nc.vector.tensor_sub(sqd[:], t4[:], e4[:])
        nc.vector.tensor_mul(sqd[:], sqd[:], sqd[:])

        # inv_s = (max-min) * recip(sq - min)
        mxp2 = pool.tile([128, 1], f32, tag="mxp2")
        nc.vector.tensor_reduce(mxp2[:], sqd[:], axis=AX.X, op=AL.max)
        mnp2 = pool.tile([128, 1], f32, tag="mnp2")
        nc.vector.tensor_reduce(mnp2[:], sqd[:], axis=AX.X, op=AL.min)
        mx216, _ = row_combine(mxp2, AL.max, "s1")
        mn216, mnbc2 = row_combine(mnp2, AL.min, "s2")
        num16 = pool.tile([16, 1], f32, tag="num16")
        nc.vector.tensor_sub(num16[:], mx216[:], mn216[:])
        numbc = psC([128, 1])
        nc.tensor.matmul(numbc[:], cs["E16b"][:], num16[:], start=True, stop=True)
        den2 = pool.tile([128, 256], f32, tag="den2")
        nc.vector.tensor_scalar(den2[:], sqd[:], mnbc2[:], None, AL.subtract)
        nc.vector.reciprocal(den2[:], den2[:])
        inv_s = pool.tile([128, 256], f32, tag="inv_s")
        nc.vector.tensor_scalar(inv_s[:], den2[:], numbc[:], None, AL.mult)

        # ln(r) (packed) and g
        rp = pool.tile([128, 256], f32, tag="rp")
        nc.sync.dma_start(rp[:], r_d.rearrange("b (q j) -> (b q) j", q=NQ)[:])
        lnr = pool.tile([128, 256], f32, tag="lnr")
        bits = rp.bitcast(i32)
        e_i = pool.tile([128, 256], i32, tag="ln_ei")
        nc.vector.tensor_scalar(e_i[:], bits[:], 23, None, AL.logical_shift_right)
        nc.vector.tensor_scalar(e_i[:], e_i[:], 127, None, AL.subtract)
        m_i = pool.tile([128, 256], i32, tag="ln_mi")
        nc.vector.tensor_scalar(m_i[:], bits[:], 0x7FFFFF, None, AL.bitwise_and)
        nc.vector.tensor_scalar(m_i[:], m_i[:], 0x3F800000, None, AL.bitwise_or)
        mf = m_i.bitcast(f32)
        e_f = pool.tile([128, 256], f32, tag="ln_ef")
        nc.vector.tensor_copy(e_f[:], e_i[:])
        sel = pool.tile([128, 256], f32, tag="ln_sel")
        nc.vector.tensor_scalar(sel[:], mf[:], SQRT2, None, AL.is_ge)
        mh = pool.tile([128, 256], f32, tag="ln_mh")
        nc.vector.tensor_scalar(mh[:], mf[:], 0.5, None, AL.mult)
        nc.vector.tensor_mul(mh[:], mh[:], sel[:])
        mm = pool.tile([128, 256], f32, tag="ln_mm")
        nc.vector.tensor_sub(mm[:], mf[:], mh[:])
        nc.vector.tensor_add(e_f[:], e_f[:], sel[:])
        lnum = pool.tile([128, 256], f32, tag="ln_nm")
        nc.vector.tensor_scalar(lnum[:], mm[:], 1.0, None, AL.subtract)
        lden = pool.tile([128, 256], f32, tag="ln_dn")
        nc.vector.tensor_scalar(lden[:], mm[:], 1.0, None, AL.add)
        nc.vector.reciprocal(lden[:], lden[:])
        tq = pool.tile([128, 256], f32, tag="ln_t")
        nc.vector.tensor_mul(tq[:], lnum[:], lden[:])
        tq2 = pool.tile([128, 256], f32, tag="ln_t2")
        nc.vector.tensor_mul(tq2[:], tq[:], tq[:])
        acc = pool.tile([128, 256], f32, tag="ln_ac")
        nc.vector.tensor_scalar(acc[:], tq2[:], float(np.float32(2.0 / 11.0)),
                                float(np.float32(2.0 / 9.0)), AL.mult, op1=AL.add)
        nc.vector.tensor_mul(acc[:], acc[:], tq2[:])
        nc.vector.tensor_scalar(acc[:], acc[:], float(np.float32(2.0 / 7.0)), None, AL.add)
        nc.vector.tensor_mul(acc[:], acc[:], tq2[:])
        nc.vector.tensor_scalar(acc[:], acc[:], float(np.float32(2.0 / 5.0)), None, AL.add)
        nc.vector.tensor_mul(acc[:], acc[:], tq2[:])
        nc.vector.tensor_scalar(acc[:], acc[:], float(np.float32(2.0 / 3.0)), None, AL.add)
        nc.vector.tensor_mul(tq2[:], tq[:], tq2[:])
        nc.vector.tensor_mul(acc[:], tq2[:], acc[:])
        nc.vector.tensor_scalar(tq[:], tq[:], 2.0, None, AL.mult)
        nc.vector.tensor_add(acc[:], tq[:], acc[:])
        nc.vector.tensor_scalar(lnum[:], e_f[:], LN2_LO, None, AL.mult)
        nc.vector.tensor_add(acc[:], acc[:], lnum[:])
        nc.vector.tensor_scalar(lnum[:], e_f[:], LN2_HI, None, AL.mult)
        nc.vector.tensor_add(lnr[:], acc[:], lnum[:])

        g = pool.tile([128, 256], f32, tag="g")
        nc.vector.tensor_mul(g[:], lnr[:], inv_s[:])

        # binary search for (k+1)-th largest g per row
        lo = pool.tile([16, 1], f32, tag="s_lo", bufs=2)
        nc.gpsimd.memset(lo[:], LO0)
        hi = pool.tile([16, 1], f32, tag="s_hi", bufs=2)
        nc.gpsimd.memset(hi[:], HI0)
        cjunk = pool.tile([128, 256], f32, tag="cjunk")
        for it in range(SEARCH_ROUNDS):
            mid = pool.tile([16, 1], f32, tag="s_mid")
            nc.vector.tensor_add(mid[:], lo[:], hi[:])
            nc.vector.tensor_scalar(mid[:], mid[:], 0.5, None, AL.mult)
            midbc = psC([128, 1])
            nc.tensor.matmul(midbc[:], cs["E16b"][:], mid[:], start=True, stop=True)
            cnt = pool.tile([128, 1], f32, tag="s_cnt")
            nc.vector.tensor_scalar(cjunk[:], g[:], midbc[:], None, AL.is_gt,
                                    op1=AL.add, accum_out=cnt[:])
            c16ps = psC([16, 1])
            nc.tensor.matmul(c16ps[:], cs["E16c"][:], cnt[:], start=True, stop=True)
            flag = pool.tile([16, 1], f32, tag="s_flag")
            nc.vector.tensor_scalar(flag[:], c16ps[:], KF, None, AL.is_gt)
            # if count > k: lo = mid else hi = mid
            dlt = pool.tile([16, 1], f32, tag="s_dlt")
            nc.vector.tensor_sub(dlt[:], mid[:], lo[:])
            nc.vector.tensor_mul(dlt[:], dlt[:], flag[:])
            lo2 = pool.tile([16, 1], f32, tag="s_lo", name=f"slo_{next(_n)}", bufs=2)
            nc.vector.tensor_add(lo2[:], lo[:], dlt[:])
            dlt2 = pool.tile([16, 1], f32, tag="s_dlt2")
            nc.vector.tensor_sub(dlt2[:], hi[:], mid[:])
            nc.vector.tensor_mul(dlt2[:], dlt2[:], flag[:])
            hi2 = pool.tile([16, 1], f32, tag="s_hi", name=f"shi_{next(_n)}", bufs=2)
            nc.vector.tensor_add(hi2[:], mid[:], dlt2[:])
            lo, hi = lo2, hi2

        # thr = rowmax(g where g <= hi)
        hibc = psC([128, 1])
        nc.tensor.matmul(hibc[:], cs["E16b"][:], hi[:], start=True, stop=True)
        selm = pool.tile([128, 256], u8, tag="selm")
        nc.vector.tensor_scalar(selm[:], g[:], hibc[:], None, AL.is_le)
        gm = pool.tile([128, 256], f32, tag="gm")
        nc.gpsimd.memset(gm[:], -1.0e38)
        nc.vector.copy_predicated(gm[:], selm[:], g[:])
        gmx = pool.tile([128, 1], f32, tag="gmx")
        nc.vector.tensor_reduce(gmx[:], gm[:], axis=AX.X, op=AL.max)
        thr16, thrbc = row_combine(gmx, AL.max, "th")

        # final mask + per-row counts
        mask01 = pool.tile([128, 256], f32, tag="mask01")
        nc.vector.tensor_scalar(mask01[:], g[:], thrbc[:], None, AL.is_le)
        cnt_f = pool.tile([128, 1], f32, tag="cnt_f")
        nc.vector.tensor_scalar(cjunk[:], g[:], thrbc[:], None, AL.is_gt,
                                op1=AL.add, accum_out=cnt_f[:])
        cab = psC([16, 1])
        nc.tensor.matmul(cab[:], cs["E16c"][:], cnt_f[:], start=True, stop=True)
        csb = sbuf_copy(cab, "csb")
        ones16 = pool.tile([16, 1], f32, tag="ones16")
        nc.vector.tensor_scalar(ones16[:], csb[:], 0.0, 1.0, AL.mult, op1=AL.add)
        totp = psC([1, 1])
        nc.tensor.matmul(totp[:], csb[:], ones16[:], start=True, stop=True)
        tot_above = sbuf_copy(totp, "tot_above")

        # AllGather per-core masked-out counts; scale = 262144 / mask_sum
        cnt_in = dram.tile([1, 1], f32, tag="cnt_in")
        cnt_out = dram.tile([NCORES, 1], f32, tag="cnt_out")
        nc.sync.dma_start(cnt_in[:], tot_above[:])
        nc.gpsimd.collective_compute(
            "AllGather", AL.bypass, replica_groups=[list(range(NCORES))],
            ins=[cnt_in.opt()], outs=[cnt_out.opt()])
        allc = pool.tile([1, NCORES], f32, tag="allc")
        nc.sync.dma_start(allc[:], cnt_out.rearrange("r o -> o r")[:])
        tota = pool.tile([1, 1], f32, tag="tota")
        nc.vector.reduce_sum(tota[:], allc.rearrange("o (a r) -> o a r", a=1)[:],
                             axis=AX.X)
        msum = pool.tile([1, 1], f32, tag="msum")
        nc.vector.tensor_scalar(msum[:], tota[:], -1.0, None, AL.mult)
        nc.vector.tensor_scalar(msum[:], msum[:], 262144.0, None, AL.add)
        rms = pool.tile([1, 1], f32, tag="rms")
        nc.vector.reciprocal(rms[:], msum[:])
        scl = pool.tile([1, 1], f32, tag="scl")
        nc.vector.tensor_scalar(scl[:], rms[:], 262144.0, None, AL.mult)
        tq0 = pool.tile([1, 1], f32, tag="tq0")
        nc.vector.tensor_mul(tq0[:], scl[:], msum[:])
        nc.vector.tensor_scalar(tq0[:], tq0[:], -1.0, None, AL.mult)
        nc.vector.tensor_scalar(tq0[:], tq0[:], 262144.0, None, AL.add)
        nc.vector.tensor_mul(tq0[:], tq0[:], rms[:])
        nc.vector.tensor_add(scl[:], scl[:], tq0[:])
        sclbc = psC([128, 1])
        nc.tensor.matmul(sclbc[:], cs["ones1"][:], scl[:], start=True, stop=True)
        sclS = sbuf_copy(sclbc, "sclS")

        # mask columns for phase C: T0/T1 [128, 128]
        smt_list = []
        for h in range(2):
            tph = psB([128, 128])
            nc.tensor.transpose(tph[:], mask01[:, h * 128:(h + 1) * 128], ident[:])
            sm = pool.tile([128, 128], f32, tag=f"smT{h}")
            nc.scalar.copy(sm[:], tph[:])
            nc.vector.tensor_scalar(sm[:], sm[:], sclS[:], None, AL.mult)
            smt_list.append(sm)

        # =========== PHASE C ===========
        nb = 0
        for i in range(NT):
            for bg in range(BL // 4):
                xt = cpool.tile([128, 4, HW], f32, tag="xc")
                src = x_d[bg * 4:(bg + 1) * 4, i * 128:(i + 1) * 128, :]
                nc.sync.dma_start(xt[:], src.rearrange("b c h -> c b h")[:])
                ot = cpool.tile([128, 4, HW], f32, tag="oc")
                for j in range(4):
                    b_loc = bg * 4 + j
                    col = b_loc * 8 + i // 2
                    smcol = smt_list[i % 2][:, col:col + 1]
                    if nb % 4 == 3:
                        nc.scalar.activation(ot[:, j, :], xt[:, j, :], AF.Copy,
                                             scale=smcol)
                    else:
                        nc.vector.tensor_scalar(ot[:, j, :], xt[:, j, :], smcol,
                                                None, AL.mult)
                    nb += 1
                dst = out_d[bg * 4:(bg + 1) * 4, i * 128:(i + 1) * 128, :]
                nc.sync.dma_start(dst.rearrange("b c h -> c b h")[:], ot[:])

    nc.finalize()
    return nc


def kernel(x, r, ratio, rho):
    x = np.ascontiguousarray(np.asarray(x, dtype=np.float32))
    r = np.ascontiguousarray(np.asarray(r, dtype=np.float32))
    ratio_f = float(np.asarray(ratio))
    rho_f = float(np.asarray(rho))
    k = int(ratio_f * C)
    key = (k, np.float32(rho_f).tobytes())
    if key not in _CACHE:
        _CACHE[key] = build(k, rho_f)
    nc = _CACHE[key]

    consts = _consts()
    xr = x.reshape(B, C, HW)
    in_maps = []
    for c in range(NCORES):
        m = {"x": np.ascontiguousarray(xr[c * BL:(c + 1) * BL]),
             "r": np.ascontiguousarray(r[c * BL:(c + 1) * BL])}
        m.update(consts)
        in_maps.append(m)
    res = run_bass_kernel_spmd(nc, in_maps, core_ids=list(range(NCORES)),
                               tmpdir=os.environ.get("BASS_TMPDIR"))
    LAST["res"] = res
    out = np.concatenate([res.results[c]["out"].reshape(BL, C, HW)
                          for c in range(NCORES)], axis=0)
    return out.reshape(B, C, 14, 14)



# revision 6
# speedup vs baseline: 1.9984x; 1.9984x over previous
"""DgCD forward (topk channel masking) on 8 Trainium2 NeuronCores.

v2: fully sharded middle + SBUF-cached x.
  - Phase A: per-row channel-block loads ([128p, 16*196], contiguous 12.5KB
    lines), avg-pool reduce, bf16 x-cache in SBUF.
  - Middle: each core computes scores/top-k only for its own 16 batch rows in
    the packed [(16b x 8q), 256] layout; cross-core coupling via 4 small
    collectives (env-pair sums, total sums, gram diag, mask count), per the
    batch%32 environment structure.
  - Phase C: mask-multiply from the bf16 cache, store only (no x re-read).
Channel order inside the middle is block-permuted (c' = (c%16)*128 + c//16);
all middle math is channel-permutation-equivariant, and phase C maps the mask
back through the same permutation.
"""
import os
import sys
sys.path.insert(0, "/opt/trn_rl_repo")
import numpy as np
from contextlib import ExitStack

import concourse.bass as bass
import concourse.bacc as bacc_mod
import concourse.mybir as mybir
import concourse.tile as tile
from concourse.bass_utils import run_bass_kernel_spmd

f32 = mybir.dt.float32
bf16 = mybir.dt.bfloat16
u8 = mybir.dt.uint8
AL = mybir.AluOpType
AF = mybir.ActivationFunctionType
AX = mybir.AxisListType

B, C, HW = 128, 2048, 196
NCORES = 8
BL = B // NCORES          # 16 batch rows per core
NJ = 16                   # sub-channels per partition block
NQ = 8                    # 256-wide packed chunks per row
FREE_B = NJ * HW          # 3136 floats per partition per row
NMID = 7                  # thresholds probed per search round
SEARCH_ROUNDS = 7         # 8^7 = 2^21 bracket shrink
LO0 = -104.0

C196 = float(np.float32(1.0 / 196.0))
C31 = float(np.float32(1.0 / 31.0))
C127 = float(np.float32(1.0 / 127.0))

_CACHE = {}
LAST = {}


def _consts():
    ident = np.eye(128, dtype=np.float32)
    E16b = np.zeros((16, 128), np.float32)   # [16,1] row vals -> [128,1] bcast
    E16c = np.zeros((128, 16), np.float32)   # per-partition -> per-row combine
    for p in range(128):
        E16b[p // 8, p] = 1.0
        E16c[p, p // 8] = 1.0
    Eh0 = np.zeros((16, 128), np.float32)    # statT [16,128] -> packed halves
    Eh1 = np.zeros((16, 128), np.float32)
    for p in range(128):
        Eh0[2 * (p % 8), p] = 1.0
        Eh1[2 * (p % 8) + 1, p] = 1.0
    E8s = np.zeros((128, 8), np.float32)     # sum over b for fixed q
    E8b = np.zeros((8, 128), np.float32)     # [8,256] chunk stats -> [128,256]
    for p in range(128):
        E8s[p, p % 8] = 1.0
        E8b[p % 8, p] = 1.0
    ones1 = np.ones((1, 128), np.float32)
    K7 = np.zeros((16, NMID), np.float32)
    for i in range(NMID):
        K7[:, i] = float(i + 1)
    return {"ident": ident, "E16b": E16b, "E16c": E16c, "Eh0": Eh0,
            "Eh1": Eh1, "E8s": E8s, "E8b": E8b, "ones1": ones1, "K7": K7}


def build(k, rho):
    nc = bacc_mod.Bacc()
    x_d = nc.dram_tensor("x", [BL, C, HW], f32, kind="ExternalInput")
    r_d = nc.dram_tensor("r", [BL, C], f32, kind="ExternalInput")
    cd = {n: nc.dram_tensor(n, list(v.shape), f32, kind="ExternalInput")
          for n, v in _consts().items()}
    out_d = nc.dram_tensor("out", [BL, C, HW], f32, kind="ExternalOutput")

    RHO = float(np.float32(rho))
    KF = float(k)

    with tile.TileContext(nc) as tc, ExitStack() as ctx:
        pool = ctx.enter_context(tc.tile_pool(name="main", bufs=1))
        big = ctx.enter_context(tc.tile_pool(name="bigp", bufs=1))
        psum = ctx.enter_context(tc.tile_pool(name="psum", bufs=1, space="PSUM"))
        dram = ctx.enter_context(tc.tile_pool(name="dram", bufs=1, space="DRAM"))
        xpool = ctx.enter_context(tc.tile_pool(name="xio", bufs=2))

        _n = iter(range(100000))

        def psB(shape):
            return psum.tile(shape, f32, tag="psB", bufs=4,
                             name=f"psB_{next(_n)}", padded_shape=[128, 256])

        def psC(shape):
            return psum.tile(shape, f32, tag="psC", bufs=4,
                             name=f"psC_{next(_n)}", padded_shape=[128, 1])

        def mid(tag, shape=None, dt=f32):
            return pool.tile(shape or [128, 256], dt, tag=tag,
                             name=f"{tag}_{next(_n)}")

        # ---- constants ----
        cs = {}
        for n, v in _consts().items():
            cs[n] = pool.tile(list(v.shape), f32, tag="c_" + n, name="c_" + n)
            nc.gpsimd.dma_start(cs[n][:], cd[n][:])
        ident = cs["ident"]

        def sbuf_copy(ps, tag, bufs=1):
            t = pool.tile([ps.shape[0], ps.shape[1]], f32, tag=tag,
                          name=f"sc_{tag}_{next(_n)}", bufs=bufs)
            nc.scalar.copy(t[:], ps[:])
            return t

        # =========== PHASE A: load x, avg-pool, cache bf16 ===========
        r_s = pool.tile([BL, C], f32, tag="rp16", bufs=2, name="r_s")
        nc.sync.dma_start(r_s[:], r_d[:])

        cache = big.tile([128, BL * FREE_B], bf16, tag="xcache")
        avgw = pool.tile([128, NJ * BL], f32, tag="avgw")   # free = (jj, b)
        xv = x_d.rearrange("b (p jj) h -> b p (jj h)", p=128)
        for b in range(BL):
            xt = xpool.tile([128, FREE_B], f32, tag="xa")
            nc.sync.dma_start(xt[:], xv[b, :, :])
            nc.vector.reduce_sum(
                avgw.rearrange("p (jj b) -> p jj b", b=BL)[:, :, b],
                xt.rearrange("p (jj h) -> p jj h", jj=NJ)[:], axis=AX.X)
            nc.scalar.activation(cache[:, b * FREE_B:(b + 1) * FREE_B], xt[:],
                                 AF.Copy)

        # ---- per-channel batch sums -> collectives ----
        nc.vector.tensor_scalar(avgw[:], avgw[:], C196, None, AL.mult)
        sqw = pool.tile([128, NJ * BL], f32, tag="sqw")
        nc.vector.tensor_mul(sqw[:], avgw[:], avgw[:])
        st = pool.tile([128, 32], f32, tag="st")
        nc.vector.reduce_sum(st[:, 0:16],
                             avgw.rearrange("p (jj b) -> p jj b", b=BL)[:],
                             axis=AX.X)
        nc.vector.reduce_sum(st[:, 16:32],
                             sqw.rearrange("p (jj b) -> p jj b", b=BL)[:],
                             axis=AX.X)
        st_in = dram.tile([128, 32], f32, tag="st_in")
        nc.sync.dma_start(st_in[:], st[:])
        st_env_d = dram.tile([128, 32], f32, tag="st_env_d")
        st_tot_d = dram.tile([128, 32], f32, tag="st_tot_d")
        nc.gpsimd.collective_compute(
            "AllReduce", AL.add, replica_groups=[[0, 1], [2, 3], [4, 5], [6, 7]],
            ins=[st_in.opt()], outs=[st_env_d.opt()])
        nc.gpsimd.collective_compute(
            "AllReduce", AL.add, replica_groups=[list(range(NCORES))],
            ins=[st_in.opt()], outs=[st_tot_d.opt()])

        # ---- r -> permuted packed + ln(r)  (overlaps collectives) ----
        r_rp = pool.tile([BL, C], f32, tag="rp16", bufs=2, name="r_rp")
        nc.vector.tensor_copy(r_rp.rearrange("b (jj p) -> b jj p", jj=NJ)[:],
                              r_s.rearrange("b (p jj) -> b jj p", jj=NJ)[:])
        r_rt = dram.tile([BL, C], f32, tag="r_rt")
        nc.sync.dma_start(r_rt[:], r_rp[:])
        r_pk = mid("r_pk")
        nc.sync.dma_start(r_pk[:], r_rt.rearrange("b (q j) -> (b q) j", q=NQ)[:])
        lnr = mid("lnr")
        nc.scalar.activation(lnr[:], r_pk[:], AF.Ln)

        # ---- avg -> row-permuted -> packed  (overlaps collectives) ----
        avg_rp = pool.tile([BL, C], f32, tag="rp16", bufs=2, name="avg_rp")
        for jj in range(NJ):
            tp = psB([BL, 128])
            nc.tensor.transpose(tp[:], avgw[:, jj * BL:(jj + 1) * BL], ident[:])
            nc.vector.tensor_copy(avg_rp[:, jj * 128:(jj + 1) * 128], tp[:])
        avg_rt = dram.tile([BL, C], f32, tag="avg_rt")
        nc.sync.dma_start(avg_rt[:], avg_rp[:])
        avg_pk = mid("avg_pk")
        nc.sync.dma_start(avg_pk[:],
                          avg_rt.rearrange("b (q j) -> (b q) j", q=NQ)[:])

        # =========== stats -> z ===========
        st_env = pool.tile([128, 32], f32, tag="st_env")
        nc.sync.dma_start(st_env[:], st_env_d[:])
        st_tot = pool.tile([128, 32], f32, tag="st_tot")
        nc.sync.dma_start(st_tot[:], st_tot_d[:])

        stats4 = pool.tile([128, 64], f32, tag="stats4")  # m_e|rsd_e|m_t|rsd_t

        def mk_stats(src, dst_m, dst_r, n, cinv):
            nc.vector.tensor_scalar(dst_m, src[:, 0:16], 1.0 / n, None, AL.mult)
            t = mid("vtmp", [128, 16])
            nc.vector.tensor_mul(t[:], dst_m, dst_m)
            nc.vector.tensor_scalar(t[:], t[:], float(n), None, AL.mult)
            v = mid("vvar", [128, 16])
            nc.vector.tensor_sub(v[:], src[:, 16:32], t[:])
            nc.vector.tensor_scalar(v[:], v[:], cinv, 1e-05, AL.mult, op1=AL.add)
            sd = mid("vsd", [128, 16])
            nc.scalar.activation(sd[:], v[:], AF.Sqrt)
            nc.vector.reciprocal(dst_r, sd[:])

        mk_stats(st_env, stats4[:, 0:16], stats4[:, 16:32], 32, C31)
        mk_stats(st_tot, stats4[:, 32:48], stats4[:, 48:64], 128, C127)

        def bcast_pk(src16, tag):
            """[128,16] per-(p,jj) channel stat -> packed [128,256] bcast."""
            tp = psB([16, 128])
            nc.tensor.transpose(tp[:], src16, ident[:])
            sT = sbuf_copy(tp, "sT_" + tag)
            ps = psB([128, 256])
            nc.tensor.matmul(ps[:, 0:128], cs["Eh0"][:], sT[:],
                             start=True, stop=True)
            nc.tensor.matmul(ps[:, 128:256], cs["Eh1"][:], sT[:],
                             start=True, stop=True)
            return sbuf_copy(ps, "bc_" + tag, bufs=2)

        m_eb = bcast_pk(stats4[:, 0:16], "me")
        rsd_eb = bcast_pk(stats4[:, 16:32], "re")
        m_tb = bcast_pk(stats4[:, 32:48], "mt")
        rsd_tb = bcast_pk(stats4[:, 48:64], "rt")

        z_e = mid("z_e")
        nc.vector.tensor_sub(z_e[:], avg_pk[:], m_eb[:])
        nc.vector.tensor_mul(z_e[:], z_e[:], rsd_eb[:])
        z_t = mid("z_t")
        nc.vector.tensor_sub(z_t[:], avg_pk[:], m_tb[:])
        nc.vector.tensor_mul(z_t[:], z_t[:], rsd_tb[:])

        # =========== packed softmax / KL / grads ===========
        def row_combine(val_pp, op, want_bc=True):
            """[128,1] per-partition -> [16,1] per-row (+[128,1] PSUM bcast)."""
            tp = psB([1, 128])
            nc.tensor.transpose(tp[:], val_pp[:], ident[:])
            s1 = sbuf_copy(tp, "rcs", bufs=2)
            red = pool.tile([1, 16], f32, tag="rcr", name=f"rcr_{next(_n)}",
                            bufs=2)
            nc.vector.tensor_reduce(red[:],
                                    s1.rearrange("o (b q) -> o b q", q=8)[:],
                                    axis=AX.X, op=op)
            tp2 = psB([16, 1])
            nc.tensor.transpose(tp2[:], red[:], ident[0:1, 0:1])
            c16 = sbuf_copy(tp2, "rc16", bufs=4)
            if not want_bc:
                return c16, None
            bc = psC([128, 1])
            nc.tensor.matmul(bc[:], cs["E16b"][:], c16[:], start=True, stop=True)
            return c16, bc

        def softmax_pk(z, sfx):
            mxp = mid("mxp" + sfx, [128, 1])
            nc.vector.tensor_reduce(mxp[:], z[:], axis=AX.X, op=AL.max)
            _, mxbc = row_combine(mxp, AL.max)
            sh = mid("sh" + sfx)
            nc.vector.tensor_scalar(sh[:], z[:], mxbc[:], None, AL.subtract)
            ex = mid("ex" + sfx)
            nc.scalar.activation(ex[:], sh[:], AF.Exp)
            esp = mid("esp" + sfx, [128, 1])
            nc.vector.reduce_sum(esp[:],
                                 ex.rearrange("p (o j) -> p o j", o=1)[:],
                                 axis=AX.X)
            es16, _ = row_combine(esp, AL.add, want_bc=False)
            ln16 = mid("ln16" + sfx, [16, 1])
            nc.scalar.activation(ln16[:], es16[:], AF.Ln)
            rs16 = mid("rs16" + sfx, [16, 1])
            nc.vector.reciprocal(rs16[:], es16[:])
            lnbc = psC([128, 1])
            nc.tensor.matmul(lnbc[:], cs["E16b"][:], ln16[:], start=True,
                             stop=True)
            rsbc = psC([128, 1])
            nc.tensor.matmul(rsbc[:], cs["E16b"][:], rs16[:], start=True,
                             stop=True)
            lsf = mid("lsf" + sfx)
            nc.vector.tensor_scalar(lsf[:], sh[:], lnbc[:], None, AL.subtract)
            p = mid("p" + sfx)
            nc.vector.tensor_scalar(p[:], ex[:], rsbc[:], None, AL.mult)
            return lsf, p

        lsf_e, p_e = softmax_pk(z_e, "e")
        lsf_t, p_t = softmax_pk(z_t, "t")

        diff = mid("diff")
        nc.vector.tensor_sub(diff[:], lsf_e[:], lsf_t[:])
        pd = mid("pd")
        nc.vector.tensor_mul(pd[:], p_e[:], diff[:])
        kp = mid("kp", [128, 1])
        nc.vector.reduce_sum(kp[:], pd.rearrange("p (o j) -> p o j", o=1)[:],
                             axis=AX.X)
        _, klbc = row_combine(kp, AL.add)
        G_env = mid("G_env")
        nc.vector.tensor_scalar(G_env[:], diff[:], klbc[:], None, AL.subtract)
        nc.vector.tensor_mul(G_env[:], p_e[:], G_env[:])
        nc.vector.tensor_scalar(G_env[:], G_env[:], 0.0078125, None, AL.mult)
        G_tot = mid("G_tot")
        nc.vector.tensor_sub(G_tot[:], p_t[:], p_e[:])
        nc.vector.tensor_scalar(G_tot[:], G_tot[:], 0.0078125, None, AL.mult)
        g_ve = mid("g_ve")
        nc.vector.tensor_mul(g_ve[:], G_env[:], z_e[:])
        g_vt = mid("g_vt")
        nc.vector.tensor_mul(g_vt[:], G_tot[:], z_t[:])

        def pert_scale(g, sfx):
            sq = mid("psq", [128, 256])
            nc.vector.tensor_mul(sq[:], g[:], g[:])
            np_ = mid("pnp" + sfx, [128, 1])
            nc.vector.reduce_sum(np_[:],
                                 sq.rearrange("p (o j) -> p o j", o=1)[:],
                                 axis=AX.X)
            n16, _ = row_combine(np_, AL.add, want_bc=False)
            nc.scalar.activation(n16[:], n16[:], AF.Sqrt)
            nc.vector.tensor_scalar(n16[:], n16[:], 1e-12, None, AL.add)
            nc.scalar.activation(n16[:], n16[:], AF.Sqrt)
            nc.vector.reciprocal(n16[:], n16[:])
            nc.vector.tensor_scalar(n16[:], n16[:], RHO, None, AL.mult)
            bc = psC([128, 1])
            nc.tensor.matmul(bc[:], cs["E16b"][:], n16[:], start=True, stop=True)
            return bc

        s_me = pert_scale(G_env, "a")
        s_ve = pert_scale(g_ve, "b")
        s_mt = pert_scale(G_tot, "c")
        s_vt = pert_scale(g_vt, "d")

        def align2(z, gm, gv, s_m, s_v, sfx):
            d = mid("d" + sfx)
            nc.vector.tensor_scalar(d[:], gv[:], s_v[:], None, AL.mult)
            nc.vector.tensor_scalar(d[:], d[:], 1.0, None, AL.add)
            out = mid("a2" + sfx)
            nc.vector.tensor_mul(out[:], z[:], d[:])
            dm = mid("dm" + sfx)
            nc.vector.tensor_scalar(dm[:], gm[:], s_m[:], None, AL.mult)
            nc.vector.tensor_add(out[:], out[:], dm[:])
            return out

        env_a2 = align2(z_e, G_env, g_ve, s_me, s_ve, "e")
        tot_a2 = align2(z_t, G_tot, g_vt, s_mt, s_vt, "t")

        # =========== gram (psum over batch) ===========
        w1 = mid("w1")
        nc.vector.tensor_scalar(w1[:], tot_a2[:], 1e-07, None, AL.add)
        w2 = mid("w2")
        nc.vector.tensor_scalar(w2[:], env_a2[:], 1e-07, None, AL.add)
        nc.vector.tensor_mul(w1[:], w1[:], w2[:])
        gps = psB([8, 256])
        nc.tensor.matmul(gps[:], cs["E8s"][:], w1[:], start=True, stop=True)
        gsb = sbuf_copy(gps, "gsb")
        gram_i = dram.tile([8, 256], f32, tag="gram_i")
        nc.sync.dma_start(gram_i[:], gsb[:])
        gram_o = dram.tile([8, 256], f32, tag="gram_o")
        nc.gpsimd.collective_compute(
            "AllReduce", AL.add, replica_groups=[list(range(NCORES))],
            ins=[gram_i.opt()], outs=[gram_o.opt()])
        gram8 = pool.tile([8, 256], f32, tag="gram8")
        nc.sync.dma_start(gram8[:], gram_o[:])
        rg8 = pool.tile([8, 256], f32, tag="rg8")
        nc.vector.reciprocal(rg8[:], gram8[:])
        rps = psB([128, 256])
        nc.tensor.matmul(rps[:], cs["E8b"][:], rg8[:], start=True, stop=True)
        rgramb = sbuf_copy(rps, "rgramb")

        t3 = mid("t3")
        nc.vector.tensor_mul(t3[:], tot_a2[:], rgramb[:])
        e3 = mid("e3")
        nc.vector.tensor_mul(e3[:], env_a2[:], rgramb[:])

        # =========== minmax -> scores -> inv_s ===========
        def minmax_pk(v, sfx):
            mxp = mid("mmx" + sfx, [128, 1])
            nc.vector.tensor_reduce(mxp[:], v[:], axis=AX.X, op=AL.max)
            mnp = mid("mmn" + sfx, [128, 1])
            nc.vector.tensor_reduce(mnp[:], v[:], axis=AX.X, op=AL.min)
            mx16, _ = row_combine(mxp, AL.max, want_bc=False)
            mn16, mnbc = row_combine(mnp, AL.min)
            num = mid("num" + sfx)
            nc.vector.tensor_scalar(num[:], v[:], mnbc[:], None, AL.subtract)
            den16 = mid("den16" + sfx, [16, 1])
            nc.vector.tensor_sub(den16[:], mx16[:], mn16[:])
            nc.vector.reciprocal(den16[:], den16[:])
            rbc = psC([128, 1])
            nc.tensor.matmul(rbc[:], cs["E16b"][:], den16[:], start=True,
                             stop=True)
            nc.vector.tensor_scalar(num[:], num[:], rbc[:], None, AL.mult)
            return num

        t4 = minmax_pk(t3, "t")
        e4 = minmax_pk(e3, "e")
        sqd = mid("sqd")
        nc.vector.tensor_sub(sqd[:], t4[:], e4[:])
        nc.vector.tensor_mul(sqd[:], sqd[:], sqd[:])

        mxp2 = mid("mxp2", [128, 1])
        nc.vector.tensor_reduce(mxp2[:], sqd[:], axis=AX.X, op=AL.max)
        mnp2 = mid("mnp2", [128, 1])
        nc.vector.tensor_reduce(mnp2[:], sqd[:], axis=AX.X, op=AL.min)
        mx216, _ = row_combine(mxp2, AL.max, want_bc=False)
        mn216, mnbc2 = row_combine(mnp2, AL.min)
        num16 = mid("num16", [16, 1])
        nc.vector.tensor_sub(num16[:], mx216[:], mn216[:])
        numbc = psC([128, 1])
        nc.tensor.matmul(numbc[:], cs["E16b"][:], num16[:], start=True, stop=True)
        den2 = mid("den2")
        nc.vector.tensor_scalar(den2[:], sqd[:], mnbc2[:], None, AL.subtract)
        nc.vector.reciprocal(den2[:], den2[:])
        inv_s = mid("inv_s")
        nc.vector.tensor_scalar(inv_s[:], den2[:], numbc[:], None, AL.mult)

        g = mid("g")
        nc.vector.tensor_mul(g[:], lnr[:], inv_s[:])

        # =========== multi-probe binary search for (k+1)-th largest ===========
        lo = pool.tile([16, 1], f32, tag="s_lo", bufs=2)
        nc.gpsimd.memset(lo[:], LO0)
        hi = pool.tile([16, 1], f32, tag="s_hi", bufs=2)
        nc.gpsimd.memset(hi[:], 0.0)
        cjunk = mid("cjunk")
        for it in range(SEARCH_ROUNDS):
            w8 = pool.tile([16, 1], f32, tag="s_w8", name=f"w8_{next(_n)}",
                           bufs=2)
            nc.vector.tensor_sub(w8[:], hi[:], lo[:])
            nc.vector.tensor_scalar(w8[:], w8[:], 0.125, None, AL.mult)
            mids = pool.tile([16, NMID], f32, tag="s_mid",
                             name=f"mids_{next(_n)}", bufs=2)
            nc.vector.tensor_scalar(mids[:], cs["K7"][:, 0:NMID], w8[:], None,
                                    AL.mult)
            nc.vector.tensor_scalar(mids[:], mids[:], lo[:], None, AL.add)
            mb = psB([128, NMID])
            nc.tensor.matmul(mb[:], cs["E16b"][:], mids[:], start=True,
                             stop=True)
            cnt7 = pool.tile([128, NMID], f32, tag="s_cnt7",
                             name=f"cnt7_{next(_n)}", bufs=2)
            for i in range(NMID):
                nc.vector.tensor_scalar(cjunk[:], g[:], mb[:, i:i + 1], None,
                                        AL.is_gt, op1=AL.add,
                                        accum_out=cnt7[:, i:i + 1])
            c16p = psB([16, NMID])
            nc.tensor.matmul(c16p[:], cs["E16c"][:], cnt7[:], start=True,
                             stop=True)
            flags = pool.tile([16, NMID], f32, tag="s_flag",
                              name=f"flag_{next(_n)}", bufs=2)
            nc.vector.tensor_scalar(flags[:], c16p[:], KF, None, AL.is_gt)
            s16 = pool.tile([16, 1], f32, tag="s_s16", name=f"s16_{next(_n)}",
                            bufs=2)
            nc.vector.reduce_sum(s16[:],
                                 flags.rearrange("p (o j) -> p o j", o=1)[:],
                                 axis=AX.X)
            step = pool.tile([16, 1], f32, tag="s_step",
                             name=f"step_{next(_n)}", bufs=2)
            nc.vector.tensor_mul(step[:], s16[:], w8[:])
            lo2 = pool.tile([16, 1], f32, tag="s_lo", name=f"lo_{next(_n)}",
                            bufs=2)
            nc.vector.tensor_add(lo2[:], lo[:], step[:])
            hi2 = pool.tile([16, 1], f32, tag="s_hi", name=f"hi_{next(_n)}",
                            bufs=2)
            nc.vector.tensor_add(hi2[:], lo2[:], w8[:])
            lo, hi = lo2, hi2

        # thr = rowmax(g where g <= hi)
        hibc = psC([128, 1])
        nc.tensor.matmul(hibc[:], cs["E16b"][:], hi[:], start=True, stop=True)
        selm = mid("selm", [128, 256], u8)
        nc.vector.tensor_scalar(selm[:], g[:], hibc[:], None, AL.is_le)
        gm = mid("gmz")
        nc.gpsimd.memset(gm[:], -1.0e38)
        nc.vector.copy_predicated(gm[:], selm[:], g[:])
        gmx = mid("gmx", [128, 1])
        nc.vector.tensor_reduce(gmx[:], gm[:], axis=AX.X, op=AL.max)
        _, thrbc = row_combine(gmx, AL.max)

        # final mask + global count -> scale
        mask01 = mid("mask01")
        nc.vector.tensor_scalar(mask01[:], g[:], thrbc[:], None, AL.is_le)
        cnt_f = mid("cnt_f", [128, 1])
        nc.vector.tensor_scalar(cjunk[:], g[:], thrbc[:], None, AL.is_gt,
                                op1=AL.add, accum_out=cnt_f[:])
        cab = psC([16, 1])
        nc.tensor.matmul(cab[:], cs["E16c"][:], cnt_f[:], start=True, stop=True)
        csb = sbuf_copy(cab, "csb")
        ones16 = pool.tile([16, 1], f32, tag="ones16")
        nc.vector.tensor_scalar(ones16[:], csb[:], 0.0, 1.0, AL.mult,
                                op1=AL.add)
        totp = psC([1, 1])
        nc.tensor.matmul(totp[:], csb[:], ones16[:], start=True, stop=True)
        tot_above = sbuf_copy(totp, "tot_above")
        cnt_i = dram.tile([1, 1], f32, tag="cnt_i")
        nc.sync.dma_start(cnt_i[:], tot_above[:])
        cnt_o = dram.tile([1, 1], f32, tag="cnt_o")
        nc.gpsimd.collective_compute(
            "AllReduce", AL.add, replica_groups=[list(range(NCORES))],
            ins=[cnt_i.opt()], outs=[cnt_o.opt()])
        tota = pool.tile([1, 1], f32, tag="tota")
        nc.sync.dma_start(tota[:], cnt_o[:])
        scl = pool.tile([1, 1], f32, tag="scl")
        nc.vector.tensor_scalar(scl[:], tota[:], -1.0, 262144.0, AL.mult,
                                op1=AL.add)
        nc.vector.reciprocal(scl[:], scl[:])
        nc.vector.tensor_scalar(scl[:], scl[:], 262144.0, None, AL.mult)
        sclbc = psC([128, 1])
        nc.tensor.matmul(sclbc[:], cs["ones1"][:], scl[:], start=True, stop=True)
        sclS = sbuf_copy(sclbc, "sclS")
        maskS = mid("maskS")
        nc.vector.tensor_scalar(maskS[:], mask01[:], sclS[:], None, AL.mult)

        # mask columns: smT_h[p, b*8+q] = scaled mask at c' = q*256+h*128+p
        smt_list = []
        for h in range(2):
            tph = psB([128, 128])
            nc.tensor.transpose(tph[:], maskS[:, h * 128:(h + 1) * 128],
                                ident[:])
            sm = pool.tile([128, 128], f32, tag=f"smT{h}")
            nc.scalar.copy(sm[:], tph[:])
            smt_list.append(sm)

        # =========== PHASE C: mask-multiply from cache, store ===========
        ov = out_d.rearrange("b (p jj) h -> b p (jj h)", p=128)
        for b in range(BL):
            ot = xpool.tile([128, FREE_B], f32, tag="xa")
            for jj in range(NJ):
                col = b * 8 + jj // 2
                smcol = smt_list[jj % 2][:, col:col + 1]
                src = cache[:, b * FREE_B + jj * HW:b * FREE_B + (jj + 1) * HW]
                dst = ot[:, jj * HW:(jj + 1) * HW]
                if jj % 4 == 3:
                    nc.scalar.activation(dst, src, AF.Copy, scale=smcol)
                else:
                    nc.vector.tensor_scalar(dst, src, smcol, None, AL.mult)
            nc.sync.dma_start(ov[b, :, :], ot[:])

    nc.finalize()
    return nc


def kernel(x, r, ratio, rho):
    x = np.ascontiguousarray(np.asarray(x, dtype=np.float32))
    r = np.ascontiguousarray(np.asarray(r, dtype=np.float32))
    ratio_f = float(np.asarray(ratio))
    rho_f = float(np.asarray(rho))
    k = int(ratio_f * C)
    key = (k, np.float32(rho_f).tobytes())
    if key not in _CACHE:
        _CACHE[key] = build(k, rho_f)
    nc = _CACHE[key]

    consts = _consts()
    xr = x.reshape(B, C, HW)
    in_maps = []
    for c in range(NCORES):
        m = {"x": np.ascontiguousarray(xr[c * BL:(c + 1) * BL]),
             "r": np.ascontiguousarray(r[c * BL:(c + 1) * BL])}
        m.update(consts)
        in_maps.append(m)
    res = run_bass_kernel_spmd(nc, in_maps, core_ids=list(range(NCORES)),
                               tmpdir=os.environ.get("BASS_TMPDIR"))
    LAST["res"] = res
    out = np.concatenate([res.results[c]["out"].reshape(BL, C, HW)
                          for c in range(NCORES)], axis=0)
    return out.reshape(B, C, 14, 14)


# revision 14
# speedup vs baseline: 2.2798x; 1.1408x over previous
"""DgCD forward (topk channel masking) on 8 Trainium2 NeuronCores.

v3: fully sharded middle + SBUF-cached x + fused combine-broadcast matmuls.
  - Phase A: per-row channel-block loads ([128p, 16*196], contiguous 12.5KB
    lines), avg-pool reduce, bf16 x-cache in SBUF.
  - Middle: each core computes scores/top-k only for its own 16 batch rows in
    the packed [(16b x 8q), 256] layout; cross-core coupling via 4 small
    collectives (env-pair sums, total sums, gram diag, mask count).
  - Row-wide sums use one EE=E16c@E16b matmul (combine+broadcast fused); the
    top-k search keeps lo/hi replicated per partition so each round is one
    matmul plus vector work. Softmax skips max-subtraction (z standardized).
  - Phase C: mask-multiply from the bf16 cache, store only (no x re-read).
Channel order in the middle is block-permuted (c' = (c%16)*128 + c//16); all
middle math is channel-permutation-equivariant and phase C maps the mask back.
"""
import os
import sys
sys.path.insert(0, "/opt/trn_rl_repo")
import numpy as np
from contextlib import ExitStack

import concourse.bass as bass
import concourse.bacc as bacc_mod
import concourse.mybir as mybir
import concourse.tile as tile
from concourse.bass_utils import run_bass_kernel_spmd

f32 = mybir.dt.float32
bf16 = mybir.dt.bfloat16
u8 = mybir.dt.uint8
AL = mybir.AluOpType
AF = mybir.ActivationFunctionType
AX = mybir.AxisListType

B, C, HW = 128, 2048, 196
NCORES = 8
BL = B // NCORES          # 16 batch rows per core
NJ = 16                   # sub-channels per partition block
NQ = 8                    # 256-wide packed chunks per row
FREE_B = NJ * HW          # 3136 floats per partition per row
NMID = 7                  # thresholds probed per search round
SEARCH_ROUNDS = 7         # 8^7 = 2^21 bracket shrink
LO0 = -104.0

C196 = float(np.float32(1.0 / 196.0))
C31 = float(np.float32(1.0 / 31.0))
C127 = float(np.float32(1.0 / 127.0))

_CACHE = {}
LAST = {}


def _consts():
    ident = np.eye(128, dtype=np.float32)
    E16b = np.zeros((16, 128), np.float32)   # [16,1] row vals -> [128,1] bcast
    for p in range(128):
        E16b[p // 8, p] = 1.0
    EE = np.zeros((128, 128), np.float32)    # row-sum + bcast in one matmul
    for kk in range(128):
        for p in range(128):
            if kk // 8 == p // 8:
                EE[kk, p] = 1.0
    Eh0 = np.zeros((16, 128), np.float32)    # statT [16,128] -> packed halves
    Eh1 = np.zeros((16, 128), np.float32)
    for p in range(128):
        Eh0[2 * (p % 8), p] = 1.0
        Eh1[2 * (p % 8) + 1, p] = 1.0
    E8s = np.zeros((128, 8), np.float32)     # sum over b for fixed q
    E8b = np.zeros((8, 128), np.float32)     # [8,256] chunk stats -> [128,256]
    for p in range(128):
        E8s[p, p % 8] = 1.0
        E8b[p % 8, p] = 1.0
    ones1 = np.ones((1, 128), np.float32)
    ones128 = np.ones((128, 1), np.float32)
    K7r = np.zeros((128, NMID), np.float32)
    for i in range(NMID):
        K7r[:, i] = float(i + 1)
    return {"ident": ident, "E16b": E16b, "EE": EE, "Eh0": Eh0, "Eh1": Eh1,
            "E8s": E8s, "E8b": E8b, "ones1": ones1, "ones128": ones128,
            "K7r": K7r}


def build(k, rho):
    nc = bacc_mod.Bacc()
    x_d = nc.dram_tensor("x", [BL, C, HW], f32, kind="ExternalInput")
    r_d = nc.dram_tensor("r", [BL, C], f32, kind="ExternalInput")
    cd = {n: nc.dram_tensor(n, list(v.shape), f32, kind="ExternalInput")
          for n, v in _consts().items()}
    out_d = nc.dram_tensor("out", [BL, C, HW], f32, kind="ExternalOutput")

    RHO = float(np.float32(rho))
    KF = float(k)

    with tile.TileContext(nc) as tc, ExitStack() as ctx:
        pool = ctx.enter_context(tc.tile_pool(name="main", bufs=1))
        big = ctx.enter_context(tc.tile_pool(name="bigp", bufs=1))
        psum = ctx.enter_context(tc.tile_pool(name="psum", bufs=1, space="PSUM"))
        dram = ctx.enter_context(tc.tile_pool(name="dram", bufs=1, space="DRAM"))
        xpool = ctx.enter_context(tc.tile_pool(name="xio", bufs=2))

        _n = iter(range(100000))

        def psB(shape):
            return psum.tile(shape, f32, tag="psB", bufs=3,
                             name=f"psB_{next(_n)}", padded_shape=[128, 256])

        def psC(shape):
            return psum.tile(shape, f32, tag="psC", bufs=4,
                             name=f"psC_{next(_n)}", padded_shape=[128, 1])

        def mid(tag, shape=None, dt=f32):
            return pool.tile(shape or [128, 256], dt, tag=tag,
                             name=f"{tag}_{next(_n)}")

        # ---- constants ----
        cs = {}
        for n, v in _consts().items():
            cs[n] = pool.tile(list(v.shape), f32, tag="c_" + n, name="c_" + n)
            nc.gpsimd.dma_start(cs[n][:], cd[n][:])
        ident = cs["ident"]

        def sbuf_copy(ps, tag, bufs=1):
            t = pool.tile([ps.shape[0], ps.shape[1]], f32, tag=tag,
                          name=f"sc_{tag}_{next(_n)}", bufs=bufs)
            nc.scalar.copy(t[:], ps[:])
            return t

        def ee_bcast(val_pp):
            """[128,1] per-partition -> per-row sums broadcast [128,1] PSUM."""
            ps = psC([128, 1])
            nc.tensor.matmul(ps[:], cs["EE"][:], val_pp[:], start=True,
                             stop=True)
            return ps

        # =========== PHASE A: load x, avg-pool, cache bf16 ===========
        r_s = pool.tile([BL, C], f32, tag="rp16", bufs=2, name="r_s")
        nc.sync.dma_start(r_s[:], r_d[:])

        cache = big.tile([128, BL * FREE_B], bf16, tag="xcache")
        avgw = pool.tile([128, NJ * BL], f32, tag="avgw")   # free = (jj, b)
        xv = x_d.rearrange("b (p jj) h -> b p (jj h)", p=128)
        for b in range(BL):
            xt = xpool.tile([128, FREE_B], f32, tag="xa")
            if b % 2 == 0:
                nc.sync.dma_start(xt[:], xv[b, :, :])
            else:
                nc.gpsimd.dma_start(xt[:], xv[b, :, :])
            nc.vector.reduce_sum(
                avgw.rearrange("p (jj b) -> p jj b", b=BL)[:, :, b],
                xt.rearrange("p (jj h) -> p jj h", jj=NJ)[:], axis=AX.X)
            nc.scalar.activation(cache[:, b * FREE_B:(b + 1) * FREE_B], xt[:],
                                 AF.Copy)

        # ---- per-channel batch sums -> collectives ----
        nc.vector.tensor_scalar(avgw[:], avgw[:], C196, None, AL.mult)
        sqw = pool.tile([128, NJ * BL], f32, tag="sqw")
        nc.vector.tensor_mul(sqw[:], avgw[:], avgw[:])
        st = pool.tile([128, 32], f32, tag="st")
        nc.vector.reduce_sum(st[:, 0:16],
                             avgw.rearrange("p (jj b) -> p jj b", b=BL)[:],
                             axis=AX.X)
        nc.vector.reduce_sum(st[:, 16:32],
                             sqw.rearrange("p (jj b) -> p jj b", b=BL)[:],
                             axis=AX.X)
        st_in = dram.tile([128, 32], f32, tag="st_in")
        nc.sync.dma_start(st_in[:], st[:])
        st_env_d = dram.tile([128, 32], f32, tag="st_env_d")
        st_tot_d = dram.tile([128, 32], f32, tag="st_tot_d")
        nc.gpsimd.collective_compute(
            "AllReduce", AL.add, replica_groups=[[0, 1], [2, 3], [4, 5], [6, 7]],
            ins=[st_in.opt()], outs=[st_env_d.opt()])
        nc.gpsimd.collective_compute(
            "AllReduce", AL.add, replica_groups=[list(range(NCORES))],
            ins=[st_in.opt()], outs=[st_tot_d.opt()])

        # ---- r -> permuted packed + ln(r)  (overlaps collectives) ----
        r_rp = pool.tile([BL, C], f32, tag="rp16", bufs=2, name="r_rp")
        nc.vector.tensor_copy(r_rp.rearrange("b (jj p) -> b jj p", jj=NJ)[:],
                              r_s.rearrange("b (p jj) -> b jj p", jj=NJ)[:])
        r_rt = dram.tile([BL, C], f32, tag="r_rt")
        nc.sync.dma_start(r_rt[:], r_rp[:])
        r_pk = mid("r_pk")
        nc.sync.dma_start(r_pk[:], r_rt.rearrange("b (q j) -> (b q) j", q=NQ)[:])
        lnr = mid("lnr")
        nc.scalar.activation(lnr[:], r_pk[:], AF.Ln)

        # ---- avg -> row-permuted -> packed  (overlaps collectives) ----
        avg_rp = pool.tile([BL, C], f32, tag="rp16", bufs=2, name="avg_rp")
        for jj in range(NJ):
            tp = psB([BL, 128])
            nc.tensor.transpose(tp[:], avgw[:, jj * BL:(jj + 1) * BL], ident[:])
            nc.vector.tensor_copy(avg_rp[:, jj * 128:(jj + 1) * 128], tp[:])
        avg_rt = dram.tile([BL, C], f32, tag="avg_rt")
        nc.sync.dma_start(avg_rt[:], avg_rp[:])
        avg_pk = mid("avg_pk")
        nc.sync.dma_start(avg_pk[:],
                          avg_rt.rearrange("b (q j) -> (b q) j", q=NQ)[:])

        # =========== stats -> z ===========
        st_env = pool.tile([128, 32], f32, tag="st_env")
        nc.sync.dma_start(st_env[:], st_env_d[:])
        st_tot = pool.tile([128, 32], f32, tag="st_tot")
        nc.sync.dma_start(st_tot[:], st_tot_d[:])

        stats4 = pool.tile([128, 64], f32, tag="stats4")  # m_e|rsd_e|m_t|rsd_t

        def mk_stats(src, dst_m, dst_r, n, cinv):
            nc.vector.tensor_scalar(dst_m, src[:, 0:16], 1.0 / n, None, AL.mult)
            t = mid("vtmp", [128, 16])
            nc.vector.tensor_mul(t[:], dst_m, dst_m)
            nc.vector.tensor_scalar(t[:], t[:], float(n), None, AL.mult)
            v = mid("vvar", [128, 16])
            nc.vector.tensor_sub(v[:], src[:, 16:32], t[:])
            nc.vector.tensor_scalar(v[:], v[:], cinv, 1e-05, AL.mult, op1=AL.add)
            sd = mid("vsd", [128, 16])
            nc.scalar.activation(sd[:], v[:], AF.Sqrt)
            nc.vector.reciprocal(dst_r, sd[:])

        mk_stats(st_env, stats4[:, 0:16], stats4[:, 16:32], 32, C31)
        mk_stats(st_tot, stats4[:, 32:48], stats4[:, 48:64], 128, C127)

        def bcast_pk(src16, tag):
            """[128,16] per-(p,jj) channel stat -> packed [128,256] bcast."""
            tp = psB([16, 128])
            nc.tensor.transpose(tp[:], src16, ident[:])
            sT = sbuf_copy(tp, "sT_" + tag)
            ps = psB([128, 256])
            nc.tensor.matmul(ps[:, 0:128], cs["Eh0"][:], sT[:],
                             start=True, stop=True)
            nc.tensor.matmul(ps[:, 128:256], cs["Eh1"][:], sT[:],
                             start=True, stop=True)
            return sbuf_copy(ps, "bc_" + tag, bufs=2)

        m_eb = bcast_pk(stats4[:, 0:16], "me")
        rsd_eb = bcast_pk(stats4[:, 16:32], "re")
        m_tb = bcast_pk(stats4[:, 32:48], "mt")
        rsd_tb = bcast_pk(stats4[:, 48:64], "rt")

        z_e = mid("z_e")
        nc.vector.tensor_sub(z_e[:], avg_pk[:], m_eb[:])
        nc.vector.tensor_mul(z_e[:], z_e[:], rsd_eb[:])
        z_t = mid("z_t")
        nc.vector.tensor_sub(z_t[:], avg_pk[:], m_tb[:])
        nc.vector.tensor_mul(z_t[:], z_t[:], rsd_tb[:])

        # ===== packed softmax (no max-shift; z is standardized) =====
        ex_e = mid("ex_e")
        nc.scalar.activation(ex_e[:], z_e[:], AF.Exp)
        ex_t = mid("ex_t")
        nc.scalar.activation(ex_t[:], z_t[:], AF.Exp)

        def softmax_tail(z, ex, sfx):
            esp = mid("esp" + sfx, [128, 1])
            nc.vector.reduce_sum(esp[:],
                                 ex.rearrange("p (o j) -> p o j", o=1)[:],
                                 axis=AX.X)
            esbc = ee_bcast(esp)
            ln128 = mid("ln128" + sfx, [128, 1])
            nc.scalar.activation(ln128[:], esbc[:], AF.Ln)
            rs128 = mid("rs128" + sfx, [128, 1])
            nc.vector.reciprocal(rs128[:], esbc[:])
            lsf = mid("lsf" + sfx)
            nc.vector.tensor_scalar(lsf[:], z[:], ln128[:], None, AL.subtract)
            p = mid("p" + sfx)
            nc.vector.tensor_scalar(p[:], ex[:], rs128[:], None, AL.mult)
            return lsf, p

        lsf_e, p_e = softmax_tail(z_e, ex_e, "e")
        lsf_t, p_t = softmax_tail(z_t, ex_t, "t")

        diff = mid("diff")
        nc.vector.tensor_sub(diff[:], lsf_e[:], lsf_t[:])
        pd = mid("pd")
        nc.vector.tensor_mul(pd[:], p_e[:], diff[:])
        kp = mid("kp", [128, 1])
        nc.vector.reduce_sum(kp[:], pd.rearrange("p (o j) -> p o j", o=1)[:],
                             axis=AX.X)
        klbc = ee_bcast(kp)
        G_env = mid("G_env")
        nc.vector.tensor_scalar(G_env[:], diff[:], klbc[:], None, AL.subtract)
        nc.vector.tensor_mul(G_env[:], p_e[:], G_env[:])
        nc.vector.tensor_scalar(G_env[:], G_env[:], 0.0078125, None, AL.mult)
        G_tot = mid("G_tot")
        nc.vector.tensor_sub(G_tot[:], p_t[:], p_e[:])
        nc.vector.tensor_scalar(G_tot[:], G_tot[:], 0.0078125, None, AL.mult)
        g_ve = mid("g_ve")
        nc.vector.tensor_mul(g_ve[:], G_env[:], z_e[:])
        g_vt = mid("g_vt")
        nc.vector.tensor_mul(g_vt[:], G_tot[:], z_t[:])

        def pert_scale(g, sfx):
            """rho / sqrt(||g||_row + 1e-12), replicated [128,1]."""
            sq = mid("psq", [128, 256])
            nc.vector.tensor_mul(sq[:], g[:], g[:])
            np_ = mid("pnp" + sfx, [128, 1])
            nc.vector.reduce_sum(np_[:],
                                 sq.rearrange("p (o j) -> p o j", o=1)[:],
                                 axis=AX.X)
            bc = ee_bcast(np_)
            s = mid("ps" + sfx, [128, 1])
            nc.scalar.activation(s[:], bc[:], AF.Sqrt)
            nc.vector.tensor_scalar(s[:], s[:], 1e-12, None, AL.add)
            nc.scalar.activation(s[:], s[:], AF.Sqrt)
            nc.vector.reciprocal(s[:], s[:])
            nc.vector.tensor_scalar(s[:], s[:], RHO, None, AL.mult)
            return s

        s_me = pert_scale(G_env, "a")
        s_ve = pert_scale(g_ve, "b")
        s_mt = pert_scale(G_tot, "c")
        s_vt = pert_scale(g_vt, "d")

        def align2(z, gm, gv, s_m, s_v, sfx):
            d = mid("d" + sfx)
            nc.vector.tensor_scalar(d[:], gv[:], s_v[:], None, AL.mult)
            nc.vector.tensor_scalar(d[:], d[:], 1.0, None, AL.add)
            out = mid("a2" + sfx)
            nc.vector.tensor_mul(out[:], z[:], d[:])
            dm = mid("dm" + sfx)
            nc.vector.tensor_scalar(dm[:], gm[:], s_m[:], None, AL.mult)
            nc.vector.tensor_add(out[:], out[:], dm[:])
            return out

        env_a2 = align2(z_e, G_env, g_ve, s_me, s_ve, "e")
        tot_a2 = align2(z_t, G_tot, g_vt, s_mt, s_vt, "t")

        # =========== gram (psum over batch) ===========
        w1 = mid("w1")
        nc.vector.tensor_scalar(w1[:], tot_a2[:], 1e-07, None, AL.add)
        w2 = mid("w2")
        nc.vector.tensor_scalar(w2[:], env_a2[:], 1e-07, None, AL.add)
        nc.vector.tensor_mul(w1[:], w1[:], w2[:])
        gps = psB([8, 256])
        nc.tensor.matmul(gps[:], cs["E8s"][:], w1[:], start=True, stop=True)
        gsb = sbuf_copy(gps, "gsb")
        gram_i = dram.tile([8, 256], f32, tag="gram_i")
        nc.sync.dma_start(gram_i[:], gsb[:])
        gram_o = dram.tile([8, 256], f32, tag="gram_o")
        nc.gpsimd.collective_compute(
            "AllReduce", AL.add, replica_groups=[list(range(NCORES))],
            ins=[gram_i.opt()], outs=[gram_o.opt()])
        gram8 = pool.tile([8, 256], f32, tag="gram8")
        nc.sync.dma_start(gram8[:], gram_o[:])
        rg8 = pool.tile([8, 256], f32, tag="rg8")
        nc.vector.reciprocal(rg8[:], gram8[:])
        rps = psB([128, 256])
        nc.tensor.matmul(rps[:], cs["E8b"][:], rg8[:], start=True, stop=True)
        rgramb = sbuf_copy(rps, "rgramb")

        t3 = mid("t3")
        nc.vector.tensor_mul(t3[:], tot_a2[:], rgramb[:])
        e3 = mid("e3")
        nc.vector.tensor_mul(e3[:], env_a2[:], rgramb[:])

        # =========== minmax -> scores -> inv_s ===========
        def mm_c2(v, sfx):
            """row extremes of packed v -> [16,2] sbuf (col0=max, col1=-min)."""
            vneg = mid("vneg", [128, 256])
            nc.vector.tensor_scalar(vneg[:], v[:], -1.0, None, AL.mult)
            mm2 = mid("mm2" + sfx, [128, 2])
            nc.vector.tensor_reduce(mm2[:, 0:1], v[:], axis=AX.X, op=AL.max)
            nc.vector.tensor_reduce(mm2[:, 1:2], vneg[:], axis=AX.X, op=AL.max)
            tp = psB([2, 128])
            nc.tensor.transpose(tp[:], mm2[:], ident[:])
            s2 = sbuf_copy(tp, "s2" + sfx, bufs=2)
            red2 = pool.tile([2, 16], f32, tag="red2", bufs=2,
                             name=f"red2_{next(_n)}")
            nc.vector.tensor_reduce(red2[:],
                                    s2.rearrange("t (b q) -> t b q", q=8)[:],
                                    axis=AX.X, op=AL.max)
            tp2 = psB([16, 2])
            nc.tensor.transpose(tp2[:], red2[:], ident[0:2, 0:2])
            return sbuf_copy(tp2, "c2" + sfx, bufs=2)

        def minmax_pk(v, sfx):
            c2 = mm_c2(v, sfx)
            den16 = mid("den16" + sfx, [16, 1])
            nc.vector.tensor_add(den16[:], c2[:, 0:1], c2[:, 1:2])
            nc.vector.reciprocal(den16[:], den16[:])
            mnbc = psC([128, 1])   # broadcast of -min
            nc.tensor.matmul(mnbc[:], cs["E16b"][:], c2[:, 1:2], start=True,
                             stop=True)
            rbc = psC([128, 1])
            nc.tensor.matmul(rbc[:], cs["E16b"][:], den16[:], start=True,
                             stop=True)
            num = mid("num" + sfx)
            nc.vector.tensor_scalar(num[:], v[:], mnbc[:], None, AL.add)
            nc.vector.tensor_scalar(num[:], num[:], rbc[:], None, AL.mult)
            return num

        t4 = minmax_pk(t3, "t")
        e4 = minmax_pk(e3, "e")
        sqd = mid("sqd")
        nc.vector.tensor_sub(sqd[:], t4[:], e4[:])
        nc.vector.tensor_mul(sqd[:], sqd[:], sqd[:])

        # inv_s = (rowmax-rowmin) / (sqd - rowmin)
        c2s = mm_c2(sqd, "s")
        num16 = mid("num16", [16, 1])
        nc.vector.tensor_add(num16[:], c2s[:, 0:1], c2s[:, 1:2])
        numbc = psC([128, 1])
        nc.tensor.matmul(numbc[:], cs["E16b"][:], num16[:], start=True,
                         stop=True)
        mnbc2 = psC([128, 1])   # broadcast of -min
        nc.tensor.matmul(mnbc2[:], cs["E16b"][:], c2s[:, 1:2], start=True,
                         stop=True)
        den2 = mid("den2")
        nc.vector.tensor_scalar(den2[:], sqd[:], mnbc2[:], None, AL.add)
        nc.vector.reciprocal(den2[:], den2[:])
        inv_s = mid("inv_s")
        nc.vector.tensor_scalar(inv_s[:], den2[:], numbc[:], None, AL.mult)

        g = mid("g")
        nc.vector.tensor_mul(g[:], lnr[:], inv_s[:])

        # ===== multi-probe search, state replicated per partition =====
        lo = pool.tile([128, 1], f32, tag="s_lo", bufs=2)
        nc.gpsimd.memset(lo[:], LO0)
        hi = pool.tile([128, 1], f32, tag="s_hi", bufs=2)
        nc.gpsimd.memset(hi[:], 0.0)
        cjunk = mid("cjunk")
        gjunk = mid("gjunk")
        for it in range(SEARCH_ROUNDS):
            w8 = pool.tile([128, 1], f32, tag="s_w8", name=f"w8_{next(_n)}",
                           bufs=2)
            nc.vector.tensor_sub(w8[:], hi[:], lo[:])
            nc.vector.tensor_scalar(w8[:], w8[:], 0.125, None, AL.mult)
            mids = pool.tile([128, NMID], f32, tag="s_mid",
                             name=f"mids_{next(_n)}", bufs=2)
            nc.vector.tensor_scalar(mids[:], cs["K7r"][:, 0:NMID], w8[:], None,
                                    AL.mult)
            nc.vector.tensor_scalar(mids[:], mids[:], lo[:], None, AL.add)
            cnt7 = pool.tile([128, NMID], f32, tag="s_cnt7",
                             name=f"cnt7_{next(_n)}", bufs=2)
            for i in range(NMID):
                nc.vector.tensor_scalar(cjunk[:], g[:], mids[:, i:i + 1], None,
                                        AL.is_gt, op1=AL.add,
                                        accum_out=cnt7[:, i:i + 1])
            cps = psB([128, NMID])
            nc.tensor.matmul(cps[:], cs["EE"][:], cnt7[:], start=True,
                             stop=True)
            flags = pool.tile([128, NMID], f32, tag="s_flag",
                              name=f"flag_{next(_n)}", bufs=2)
            nc.vector.tensor_scalar(flags[:], cps[:], KF, None, AL.is_gt)
            s16 = pool.tile([128, 1], f32, tag="s_s16", name=f"s16_{next(_n)}",
                            bufs=2)
            nc.vector.reduce_sum(s16[:],
                                 flags.rearrange("p (o j) -> p o j", o=1)[:],
                                 axis=AX.X)
            step = pool.tile([128, 1], f32, tag="s_step",
                             name=f"step_{next(_n)}", bufs=2)
            nc.vector.tensor_mul(step[:], s16[:], w8[:])
            lo2 = pool.tile([128, 1], f32, tag="s_lo", name=f"lo_{next(_n)}",
                            bufs=2)
            nc.vector.tensor_add(lo2[:], lo[:], step[:])
            hi2 = pool.tile([128, 1], f32, tag="s_hi", name=f"hi_{next(_n)}",
                            bufs=2)
            nc.vector.tensor_add(hi2[:], lo2[:], w8[:])
            lo, hi = lo2, hi2

        # global masked-out count -> kick collective early (overlaps thr/mask)
        cnt_f = mid("cnt_f", [128, 1])
        nc.vector.tensor_scalar(cjunk[:], g[:], hi[:], None, AL.is_gt,
                                op1=AL.add, accum_out=cnt_f[:])
        totp = psC([1, 1])
        nc.tensor.matmul(totp[:], cs["ones128"][:], cnt_f[:], start=True,
                         stop=True)
        tot_above = sbuf_copy(totp, "tot_above")
        cnt_i = dram.tile([1, 1], f32, tag="cnt_i")
        nc.sync.dma_start(cnt_i[:], tot_above[:])
        cnt_o = dram.tile([1, 1], f32, tag="cnt_o")
        nc.gpsimd.collective_compute(
            "AllReduce", AL.add, replica_groups=[list(range(NCORES))],
            ins=[cnt_i.opt()], outs=[cnt_o.opt()])

        # thr = rowmax(g where g <= hi); hi already replicated per partition
        selm = mid("selm", [128, 256], u8)
        nc.vector.tensor_scalar(selm[:], g[:], hi[:], None, AL.is_le)
        gm = mid("gmz")
        nc.gpsimd.memset(gm[:], -1.0e38)
        nc.vector.copy_predicated(gm[:], selm[:], g[:])
        gmx = mid("gmx", [128, 1])
        nc.vector.tensor_reduce(gmx[:], gm[:], axis=AX.X, op=AL.max)
        tpx = psB([1, 128])
        nc.tensor.transpose(tpx[:], gmx[:], ident[:])
        sx = sbuf_copy(tpx, "sx")
        redx = pool.tile([1, 16], f32, tag="redx")
        nc.vector.tensor_reduce(redx[:],
                                sx.rearrange("o (b q) -> o b q", q=8)[:],
                                axis=AX.X, op=AL.max)
        tpx2 = psB([16, 1])
        nc.tensor.transpose(tpx2[:], redx[:], ident[0:1, 0:1])
        thr16 = sbuf_copy(tpx2, "thr16")
        thrbc = psC([128, 1])
        nc.tensor.matmul(thrbc[:], cs["E16b"][:], thr16[:], start=True,
                         stop=True)
        mask01 = mid("mask01")
        nc.vector.tensor_scalar(mask01[:], g[:], thrbc[:], None, AL.is_le)

        # scale = 262144 / (262144 - total_above)
        tota = pool.tile([1, 1], f32, tag="tota")
        nc.sync.dma_start(tota[:], cnt_o[:])
        scl = pool.tile([1, 1], f32, tag="scl")
        nc.vector.tensor_scalar(scl[:], tota[:], -1.0, 262144.0, AL.mult,
                                op1=AL.add)
        nc.vector.reciprocal(scl[:], scl[:])
        nc.vector.tensor_scalar(scl[:], scl[:], 262144.0, None, AL.mult)
        sclbc = psC([128, 1])
        nc.tensor.matmul(sclbc[:], cs["ones1"][:], scl[:], start=True, stop=True)
        maskS = mid("maskS")
        nc.vector.tensor_scalar(maskS[:], mask01[:], sclbc[:], None, AL.mult)

        # mask columns: smT_h[p, b*8+q] = scaled mask at c' = q*256+h*128+p
        smt_list = []
        for h in range(2):
            tph = psB([128, 128])
            nc.tensor.transpose(tph[:], maskS[:, h * 128:(h + 1) * 128],
                                ident[:])
            sm = pool.tile([128, 128], f32, tag=f"smT{h}")
            nc.scalar.copy(sm[:], tph[:])
            smt_list.append(sm)

        # =========== PHASE C: mask-multiply from cache, store ===========
        ov = out_d.rearrange("b (p jj) h -> b p (jj h)", p=128)
        for b in range(BL):
            ot = xpool.tile([128, FREE_B], f32, tag="xa")
            for jj in range(NJ):
                col = b * 8 + jj // 2
                smcol = smt_list[jj % 2][:, col:col + 1]
                src = cache[:, b * FREE_B + jj * HW:b * FREE_B + (jj + 1) * HW]
                dst = ot[:, jj * HW:(jj + 1) * HW]
                if jj % 4 == 3:
                    nc.scalar.activation(dst, src, AF.Copy, scale=smcol)
                else:
                    nc.vector.tensor_scalar(dst, src, smcol, None, AL.mult)
            if b % 2 == 0:
                nc.sync.dma_start(ov[b, :, :], ot[:])
            else:
                nc.gpsimd.dma_start(ov[b, :, :], ot[:])

    nc.finalize()
    return nc


def kernel(x, r, ratio, rho):
    x = np.ascontiguousarray(np.asarray(x, dtype=np.float32))
    r = np.ascontiguousarray(np.asarray(r, dtype=np.float32))
    ratio_f = float(np.asarray(ratio))
    rho_f = float(np.asarray(rho))
    k = int(ratio_f * C)
    key = (k, np.float32(rho_f).tobytes())
    if key not in _CACHE:
        _CACHE[key] = build(k, rho_f)
    nc = _CACHE[key]

    consts = _consts()
    xr = x.reshape(B, C, HW)
    in_maps = []
    for c in range(NCORES):
        m = {"x": np.ascontiguousarray(xr[c * BL:(c + 1) * BL]),
             "r": np.ascontiguousarray(r[c * BL:(c + 1) * BL])}
        m.update(consts)
        in_maps.append(m)
    res = run_bass_kernel_spmd(nc, in_maps, core_ids=list(range(NCORES)),
                               tmpdir=os.environ.get("BASS_TMPDIR"))
    LAST["res"] = res
    out = np.concatenate([res.results[c]["out"].reshape(BL, C, HW)
                          for c in range(NCORES)], axis=0)
    return out.reshape(B, C, 14, 14)


# revision 22
# speedup vs baseline: 2.3147x; 1.0153x over previous
"""DgCD forward (topk channel masking) on 8 Trainium2 NeuronCores.

v3: fully sharded middle + SBUF-cached x + fused combine-broadcast matmuls.
  - Phase A: per-row channel-block loads ([128p, 16*196], contiguous 12.5KB
    lines), avg-pool reduce, bf16 x-cache in SBUF.
  - Middle: each core computes scores/top-k only for its own 16 batch rows in
    the packed [(16b x 8q), 256] layout; cross-core coupling via 4 small
    collectives (env-pair sums, total sums, gram diag, mask count).
  - Row-wide sums use one EE=E16c@E16b matmul (combine+broadcast fused); the
    top-k search keeps lo/hi replicated per partition so each round is one
    matmul plus vector work. Softmax skips max-subtraction (z standardized).
  - Phase C: mask-multiply from the bf16 cache, store only (no x re-read).
Channel order in the middle is block-permuted (c' = (c%16)*128 + c//16); all
middle math is channel-permutation-equivariant and phase C maps the mask back.
"""
import os
import sys
sys.path.insert(0, "/opt/trn_rl_repo")
import numpy as np
from contextlib import ExitStack

import concourse.bass as bass
import concourse.bacc as bacc_mod
import concourse.mybir as mybir
import concourse.tile as tile
from concourse.bass_utils import run_bass_kernel_spmd

f32 = mybir.dt.float32
bf16 = mybir.dt.bfloat16
u8 = mybir.dt.uint8
AL = mybir.AluOpType
AF = mybir.ActivationFunctionType
AX = mybir.AxisListType

B, C, HW = 128, 2048, 196
NCORES = 8
BL = B // NCORES          # 16 batch rows per core
NJ = 16                   # sub-channels per partition block
NQ = 8                    # 256-wide packed chunks per row
FREE_B = NJ * HW          # 3136 floats per partition per row
NMID = 7                  # thresholds probed per search round
SEARCH_ROUNDS = 6         # 8^6 = 2^18 bracket shrink
LO0 = -104.0

C196 = float(np.float32(1.0 / 196.0))
C31 = float(np.float32(1.0 / 31.0))
C127 = float(np.float32(1.0 / 127.0))

_CACHE = {}
LAST = {}


def _consts():
    ident = np.eye(128, dtype=np.float32)
    E16b = np.zeros((16, 128), np.float32)   # [16,1] row vals -> [128,1] bcast
    for p in range(128):
        E16b[p // 8, p] = 1.0
    EE = np.zeros((128, 128), np.float32)    # row-sum + bcast in one matmul
    for kk in range(128):
        for p in range(128):
            if kk // 8 == p // 8:
                EE[kk, p] = 1.0
    Eh0 = np.zeros((16, 128), np.float32)    # statT [16,128] -> packed halves
    Eh1 = np.zeros((16, 128), np.float32)
    for p in range(128):
        Eh0[2 * (p % 8), p] = 1.0
        Eh1[2 * (p % 8) + 1, p] = 1.0
    E8s = np.zeros((128, 8), np.float32)     # sum over b for fixed q
    E8b = np.zeros((8, 128), np.float32)     # [8,256] chunk stats -> [128,256]
    for p in range(128):
        E8s[p, p % 8] = 1.0
        E8b[p % 8, p] = 1.0
    ones1 = np.ones((1, 128), np.float32)
    ones128 = np.ones((128, 1), np.float32)
    K7r = np.zeros((128, NMID), np.float32)
    for i in range(NMID):
        K7r[:, i] = float(i + 1)
    return {"ident": ident, "E16b": E16b, "EE": EE, "Eh0": Eh0, "Eh1": Eh1,
            "E8s": E8s, "E8b": E8b, "ones1": ones1, "ones128": ones128,
            "K7r": K7r}


def build(k, rho):
    nc = bacc_mod.Bacc()
    x_d = nc.dram_tensor("x", [BL, C, HW], f32, kind="ExternalInput")
    r_d = nc.dram_tensor("r", [BL, C], f32, kind="ExternalInput")
    envm_d = nc.dram_tensor("envm", [128, 128], f32, kind="ExternalInput")
    cd = {n: nc.dram_tensor(n, list(v.shape), f32, kind="ExternalInput")
          for n, v in _consts().items()}
    out_d = nc.dram_tensor("out", [BL, C, HW], f32, kind="ExternalOutput")

    RHO = float(np.float32(rho))
    KF = float(k)

    with tile.TileContext(nc) as tc, ExitStack() as ctx:
        pool = ctx.enter_context(tc.tile_pool(name="main", bufs=1))
        big = ctx.enter_context(tc.tile_pool(name="bigp", bufs=1))
        psum = ctx.enter_context(tc.tile_pool(name="psum", bufs=1, space="PSUM"))
        dram = ctx.enter_context(tc.tile_pool(name="dram", bufs=1, space="DRAM"))
        xpool = ctx.enter_context(tc.tile_pool(name="xio", bufs=4))

        _n = iter(range(100000))

        def psB(shape):
            return psum.tile(shape, f32, tag="psB", bufs=3,
                             name=f"psB_{next(_n)}", padded_shape=[128, 256])

        def psC(shape):
            return psum.tile(shape, f32, tag="psC", bufs=4,
                             name=f"psC_{next(_n)}", padded_shape=[128, 1])

        def mid(tag, shape=None, dt=f32):
            return pool.tile(shape or [128, 256], dt, tag=tag,
                             name=f"{tag}_{next(_n)}")

        # ---- constants ----
        cs = {}
        for n, v in _consts().items():
            cs[n] = pool.tile(list(v.shape), f32, tag="c_" + n, name="c_" + n)
            nc.gpsimd.dma_start(cs[n][:], cd[n][:])
        ident = cs["ident"]

        def sbuf_copy(ps, tag, bufs=1):
            t = pool.tile([ps.shape[0], ps.shape[1]], f32, tag=tag,
                          name=f"sc_{tag}_{next(_n)}", bufs=bufs)
            nc.scalar.copy(t[:], ps[:])
            return t

        def ee_bcast(val_pp):
            """[128,1] per-partition -> per-row sums broadcast [128,1] PSUM."""
            ps = psC([128, 1])
            nc.tensor.matmul(ps[:], cs["EE"][:], val_pp[:], start=True,
                             stop=True)
            return ps

        # =========== PHASE A: load x, avg-pool, cache bf16 ===========
        r_s = pool.tile([BL, C], f32, tag="rp16", bufs=2, name="r_s")
        nc.sync.dma_start(r_s[:], r_d[:])

        envm = pool.tile([128, 128], f32, tag="envm")
        nc.gpsimd.dma_start(envm[:], envm_d[:])

        cache = big.tile([128, BL * FREE_B], bf16, tag="xcache")
        avgw = pool.tile([128, NJ * BL], f32, tag="avgw")   # free = (jj, b)
        HFREE = FREE_B // 2
        xv = x_d.rearrange("b (p jj) h -> b p (jj h)", p=128)
        nhalf = 0
        for b in range(BL):
            for h in range(2):
                xt = xpool.tile([128, HFREE], f32, tag="xa")
                src = xv[b, :, h * HFREE:(h + 1) * HFREE]
                if nhalf % 2 == 0:
                    nc.sync.dma_start(xt[:], src)
                else:
                    nc.gpsimd.dma_start(xt[:], src)
                nhalf += 1
                nc.vector.reduce_sum(
                    avgw.rearrange("p (jj b) -> p jj b", b=BL)
                    [:, h * 8:(h + 1) * 8, b],
                    xt.rearrange("p (jj hh) -> p jj hh", jj=NJ // 2)[:],
                    axis=AX.X)
                nc.scalar.activation(
                    cache[:, b * FREE_B + h * HFREE:b * FREE_B + (h + 1) * HFREE],
                    xt[:], AF.Copy)

        # ---- per-channel batch sums -> single merged collective ----
        nc.vector.tensor_scalar(avgw[:], avgw[:], C196, None, AL.mult)
        sqw = pool.tile([128, NJ * BL], f32, tag="sqw")
        nc.vector.tensor_mul(sqw[:], avgw[:], avgw[:])
        st = pool.tile([128, 32], f32, tag="st")
        nc.vector.reduce_sum(st[:, 0:16],
                             avgw.rearrange("p (jj b) -> p jj b", b=BL)[:],
                             axis=AX.X)
        nc.vector.reduce_sum(st[:, 16:32],
                             sqw.rearrange("p (jj b) -> p jj b", b=BL)[:],
                             axis=AX.X)
        # payload slot e (32 cols) = st if e == my env else 0
        payl = pool.tile([128, 128], f32, tag="payl")
        for e in range(4):
            nc.vector.tensor_mul(payl[:, e * 32:(e + 1) * 32], st[:],
                                 envm[:, e * 32:(e + 1) * 32])
        st_in = dram.tile([128, 128], f32, tag="st_in")
        nc.sync.dma_start(st_in[:], payl[:])
        st_out_d = dram.tile([128, 128], f32, tag="st_out_d")
        nc.gpsimd.collective_compute(
            "AllReduce", AL.add, replica_groups=[list(range(NCORES))],
            ins=[st_in.opt()], outs=[st_out_d.opt()])

        # ---- r -> permuted packed + ln(r)  (overlaps collectives) ----
        r_rp = pool.tile([BL, C], f32, tag="rp16", bufs=2, name="r_rp")
        nc.vector.tensor_copy(r_rp.rearrange("b (jj p) -> b jj p", jj=NJ)[:],
                              r_s.rearrange("b (p jj) -> b jj p", jj=NJ)[:])
        r_rt = dram.tile([BL, C], f32, tag="r_rt")
        nc.sync.dma_start(r_rt[:], r_rp[:])
        r_pk = mid("r_pk")
        nc.sync.dma_start(r_pk[:], r_rt.rearrange("b (q j) -> (b q) j", q=NQ)[:])
        lnr = mid("lnr")
        nc.scalar.activation(lnr[:], r_pk[:], AF.Ln)

        # ---- avg -> row-permuted -> packed  (overlaps collectives) ----
        avg_rp = pool.tile([BL, C], f32, tag="rp16", bufs=2, name="avg_rp")
        for jj in range(NJ):
            tp = psB([BL, 128])
            nc.tensor.transpose(tp[:], avgw[:, jj * BL:(jj + 1) * BL], ident[:])
            nc.vector.tensor_copy(avg_rp[:, jj * 128:(jj + 1) * 128], tp[:])
        avg_rt = dram.tile([BL, C], f32, tag="avg_rt")
        nc.sync.dma_start(avg_rt[:], avg_rp[:])
        avg_pk = mid("avg_pk")
        nc.sync.dma_start(avg_pk[:],
                          avg_rt.rearrange("b (q j) -> (b q) j", q=NQ)[:])

        # =========== stats -> z ===========
        st_all = pool.tile([128, 128], f32, tag="st_all")
        nc.sync.dma_start(st_all[:], st_out_d[:])
        st_tot = pool.tile([128, 32], f32, tag="st_tot")
        nc.vector.reduce_sum(st_tot[:],
                             st_all.rearrange("p (e s) -> p s e", e=4)[:],
                             axis=AX.X)
        envp = pool.tile([128, 128], f32, tag="envp")
        nc.vector.tensor_mul(envp[:], st_all[:], envm[:])
        st_env = pool.tile([128, 32], f32, tag="st_env")
        nc.vector.reduce_sum(st_env[:],
                             envp.rearrange("p (e s) -> p s e", e=4)[:],
                             axis=AX.X)

        stats4 = pool.tile([128, 64], f32, tag="stats4")  # m_e|rsd_e|m_t|rsd_t

        def mk_stats(src, dst_m, dst_r, n, cinv):
            nc.vector.tensor_scalar(dst_m, src[:, 0:16], 1.0 / n, None, AL.mult)
            t = mid("vtmp", [128, 16])
            nc.vector.tensor_mul(t[:], dst_m, dst_m)
            nc.vector.tensor_scalar(t[:], t[:], float(n), None, AL.mult)
            v = mid("vvar", [128, 16])
            nc.vector.tensor_sub(v[:], src[:, 16:32], t[:])
            nc.vector.tensor_scalar(v[:], v[:], cinv, 1e-05, AL.mult, op1=AL.add)
            sd = mid("vsd", [128, 16])
            nc.scalar.activation(sd[:], v[:], AF.Sqrt)
            nc.vector.reciprocal(dst_r, sd[:])

        mk_stats(st_env, stats4[:, 0:16], stats4[:, 16:32], 32, C31)
        mk_stats(st_tot, stats4[:, 32:48], stats4[:, 48:64], 128, C127)

        def bcast_pk(src16, tag):
            """[128,16] per-(p,jj) channel stat -> packed [128,256] bcast."""
            tp = psB([16, 128])
            nc.tensor.transpose(tp[:], src16, ident[:])
            sT = sbuf_copy(tp, "sT_" + tag)
            ps = psB([128, 256])
            nc.tensor.matmul(ps[:, 0:128], cs["Eh0"][:], sT[:],
                             start=True, stop=True)
            nc.tensor.matmul(ps[:, 128:256], cs["Eh1"][:], sT[:],
                             start=True, stop=True)
            return sbuf_copy(ps, "bc_" + tag, bufs=2)

        m_eb = bcast_pk(stats4[:, 0:16], "me")
        rsd_eb = bcast_pk(stats4[:, 16:32], "re")
        m_tb = bcast_pk(stats4[:, 32:48], "mt")
        rsd_tb = bcast_pk(stats4[:, 48:64], "rt")

        z_e = mid("z_e")
        nc.vector.tensor_sub(z_e[:], avg_pk[:], m_eb[:])
        nc.vector.tensor_mul(z_e[:], z_e[:], rsd_eb[:])
        z_t = mid("z_t")
        nc.vector.tensor_sub(z_t[:], avg_pk[:], m_tb[:])
        nc.vector.tensor_mul(z_t[:], z_t[:], rsd_tb[:])

        # ===== packed softmax (no max-shift; z is standardized) =====
        ex_e = mid("ex_e")
        nc.scalar.activation(ex_e[:], z_e[:], AF.Exp)
        ex_t = mid("ex_t")
        nc.scalar.activation(ex_t[:], z_t[:], AF.Exp)

        def softmax_tail(z, ex, sfx):
            esp = mid("esp" + sfx, [128, 1])
            nc.vector.reduce_sum(esp[:],
                                 ex.rearrange("p (o j) -> p o j", o=1)[:],
                                 axis=AX.X)
            esbc = ee_bcast(esp)
            ln128 = mid("ln128" + sfx, [128, 1])
            nc.scalar.activation(ln128[:], esbc[:], AF.Ln)
            rs128 = mid("rs128" + sfx, [128, 1])
            nc.vector.reciprocal(rs128[:], esbc[:])
            lsf = mid("lsf" + sfx)
            nc.vector.tensor_scalar(lsf[:], z[:], ln128[:], None, AL.subtract)
            p = mid("p" + sfx)
            nc.vector.tensor_scalar(p[:], ex[:], rs128[:], None, AL.mult)
            return lsf, p

        lsf_e, p_e = softmax_tail(z_e, ex_e, "e")
        lsf_t, p_t = softmax_tail(z_t, ex_t, "t")

        diff = mid("diff")
        nc.vector.tensor_sub(diff[:], lsf_e[:], lsf_t[:])
        pd = mid("pd")
        nc.vector.tensor_mul(pd[:], p_e[:], diff[:])
        kp = mid("kp", [128, 1])
        nc.vector.reduce_sum(kp[:], pd.rearrange("p (o j) -> p o j", o=1)[:],
                             axis=AX.X)
        klbc = ee_bcast(kp)
        G_env = mid("G_env")
        nc.vector.tensor_scalar(G_env[:], diff[:], klbc[:], None, AL.subtract)
        nc.vector.tensor_mul(G_env[:], p_e[:], G_env[:])
        nc.vector.tensor_scalar(G_env[:], G_env[:], 0.0078125, None, AL.mult)
        G_tot = mid("G_tot")
        nc.vector.tensor_sub(G_tot[:], p_t[:], p_e[:])
        nc.vector.tensor_scalar(G_tot[:], G_tot[:], 0.0078125, None, AL.mult)
        g_ve = mid("g_ve")
        nc.vector.tensor_mul(g_ve[:], G_env[:], z_e[:])
        g_vt = mid("g_vt")
        nc.vector.tensor_mul(g_vt[:], G_tot[:], z_t[:])

        def pert_scale(g, sfx):
            """rho / sqrt(||g||_row + 1e-12), replicated [128,1]."""
            sq = mid("psq", [128, 256])
            nc.vector.tensor_mul(sq[:], g[:], g[:])
            np_ = mid("pnp" + sfx, [128, 1])
            nc.vector.reduce_sum(np_[:],
                                 sq.rearrange("p (o j) -> p o j", o=1)[:],
                                 axis=AX.X)
            bc = ee_bcast(np_)
            s = mid("ps" + sfx, [128, 1])
            nc.scalar.activation(s[:], bc[:], AF.Sqrt)
            nc.vector.tensor_scalar(s[:], s[:], 1e-12, None, AL.add)
            nc.scalar.activation(s[:], s[:], AF.Sqrt)
            nc.vector.reciprocal(s[:], s[:])
            nc.vector.tensor_scalar(s[:], s[:], RHO, None, AL.mult)
            return s

        s_me = pert_scale(G_env, "a")
        s_ve = pert_scale(g_ve, "b")
        s_mt = pert_scale(G_tot, "c")
        s_vt = pert_scale(g_vt, "d")

        def align2(z, gm, gv, s_m, s_v, sfx):
            d = mid("d" + sfx)
            nc.vector.tensor_scalar(d[:], gv[:], s_v[:], None, AL.mult)
            nc.vector.tensor_scalar(d[:], d[:], 1.0, None, AL.add)
            out = mid("a2" + sfx)
            nc.vector.tensor_mul(out[:], z[:], d[:])
            dm = mid("dm" + sfx)
            nc.vector.tensor_scalar(dm[:], gm[:], s_m[:], None, AL.mult)
            nc.vector.tensor_add(out[:], out[:], dm[:])
            return out

        env_a2 = align2(z_e, G_env, g_ve, s_me, s_ve, "e")
        tot_a2 = align2(z_t, G_tot, g_vt, s_mt, s_vt, "t")

        # =========== gram (psum over batch) ===========
        w1 = mid("w1")
        nc.vector.tensor_scalar(w1[:], tot_a2[:], 1e-07, None, AL.add)
        w2 = mid("w2")
        nc.vector.tensor_scalar(w2[:], env_a2[:], 1e-07, None, AL.add)
        nc.vector.tensor_mul(w1[:], w1[:], w2[:])
        gps = psB([8, 256])
        nc.tensor.matmul(gps[:], cs["E8s"][:], w1[:], start=True, stop=True)
        gsb = sbuf_copy(gps, "gsb")
        gram_i = dram.tile([8, 256], f32, tag="gram_i")
        nc.sync.dma_start(gram_i[:], gsb[:])
        gram_o = dram.tile([8, 256], f32, tag="gram_o")
        nc.gpsimd.collective_compute(
            "AllReduce", AL.add, replica_groups=[list(range(NCORES))],
            ins=[gram_i.opt()], outs=[gram_o.opt()])
        gram8 = pool.tile([8, 256], f32, tag="gram8")
        nc.sync.dma_start(gram8[:], gram_o[:])
        rg8 = pool.tile([8, 256], f32, tag="rg8")
        nc.vector.reciprocal(rg8[:], gram8[:])
        rps = psB([128, 256])
        nc.tensor.matmul(rps[:], cs["E8b"][:], rg8[:], start=True, stop=True)
        rgramb = sbuf_copy(rps, "rgramb")

        t3 = mid("t3")
        nc.vector.tensor_mul(t3[:], tot_a2[:], rgramb[:])
        e3 = mid("e3")
        nc.vector.tensor_mul(e3[:], env_a2[:], rgramb[:])

        # =========== minmax -> scores -> inv_s ===========
        def mm_c2(v, sfx):
            """row extremes of packed v -> [16,2] sbuf (col0=max, col1=-min)."""
            vneg = mid("vneg", [128, 256])
            nc.vector.tensor_scalar(vneg[:], v[:], -1.0, None, AL.mult)
            mm2 = mid("mm2" + sfx, [128, 2])
            nc.vector.tensor_reduce(mm2[:, 0:1], v[:], axis=AX.X, op=AL.max)
            nc.vector.tensor_reduce(mm2[:, 1:2], vneg[:], axis=AX.X, op=AL.max)
            tp = psB([2, 128])
            nc.tensor.transpose(tp[:], mm2[:], ident[:])
            s2 = sbuf_copy(tp, "s2" + sfx, bufs=2)
            red2 = pool.tile([2, 16], f32, tag="red2", bufs=2,
                             name=f"red2_{next(_n)}")
            nc.vector.tensor_reduce(red2[:],
                                    s2.rearrange("t (b q) -> t b q", q=8)[:],
                                    axis=AX.X, op=AL.max)
            tp2 = psB([16, 2])
            nc.tensor.transpose(tp2[:], red2[:], ident[0:2, 0:2])
            return sbuf_copy(tp2, "c2" + sfx, bufs=2)

        def minmax_pk(v, sfx):
            c2 = mm_c2(v, sfx)
            den16 = mid("den16" + sfx, [16, 1])
            nc.vector.tensor_add(den16[:], c2[:, 0:1], c2[:, 1:2])
            nc.vector.reciprocal(den16[:], den16[:])
            mnbc = psC([128, 1])   # broadcast of -min
            nc.tensor.matmul(mnbc[:], cs["E16b"][:], c2[:, 1:2], start=True,
                             stop=True)
            rbc = psC([128, 1])
            nc.tensor.matmul(rbc[:], cs["E16b"][:], den16[:], start=True,
                             stop=True)
            num = mid("num" + sfx)
            nc.vector.tensor_scalar(num[:], v[:], mnbc[:], None, AL.add)
            nc.vector.tensor_scalar(num[:], num[:], rbc[:], None, AL.mult)
            return num

        t4 = minmax_pk(t3, "t")
        e4 = minmax_pk(e3, "e")
        sqd = mid("sqd")
        nc.vector.tensor_sub(sqd[:], t4[:], e4[:])
        nc.vector.tensor_mul(sqd[:], sqd[:], sqd[:])

        # inv_s = (rowmax-rowmin) / (sqd - rowmin)
        c2s = mm_c2(sqd, "s")
        num16 = mid("num16", [16, 1])
        nc.vector.tensor_add(num16[:], c2s[:, 0:1], c2s[:, 1:2])
        numbc = psC([128, 1])
        nc.tensor.matmul(numbc[:], cs["E16b"][:], num16[:], start=True,
                         stop=True)
        mnbc2 = psC([128, 1])   # broadcast of -min
        nc.tensor.matmul(mnbc2[:], cs["E16b"][:], c2s[:, 1:2], start=True,
                         stop=True)
        den2 = mid("den2")
        nc.vector.tensor_scalar(den2[:], sqd[:], mnbc2[:], None, AL.add)
        nc.vector.reciprocal(den2[:], den2[:])
        inv_s = mid("inv_s")
        nc.vector.tensor_scalar(inv_s[:], den2[:], numbc[:], None, AL.mult)

        g = mid("g")
        nc.vector.tensor_mul(g[:], lnr[:], inv_s[:])

        # ===== multi-probe search, state replicated per partition =====
        lo = pool.tile([128, 1], f32, tag="s_lo", bufs=2)
        nc.gpsimd.memset(lo[:], LO0)
        hi = pool.tile([128, 1], f32, tag="s_hi", bufs=2)
        nc.gpsimd.memset(hi[:], 0.0)
        cjunk = mid("cjunk")
        for it in range(SEARCH_ROUNDS):
            w8 = pool.tile([128, 1], f32, tag="s_w8", name=f"w8_{next(_n)}",
                           bufs=2)
            nc.vector.tensor_sub(w8[:], hi[:], lo[:])
            nc.vector.tensor_scalar(w8[:], w8[:], 0.125, None, AL.mult)
            mids = pool.tile([128, NMID], f32, tag="s_mid",
                             name=f"mids_{next(_n)}", bufs=2)
            nc.vector.tensor_scalar(mids[:], cs["K7r"][:, 0:NMID], w8[:], None,
                                    AL.mult)
            nc.vector.tensor_scalar(mids[:], mids[:], lo[:], None, AL.add)
            cnt7 = pool.tile([128, NMID], f32, tag="s_cnt7",
                             name=f"cnt7_{next(_n)}", bufs=2)
            for i in range(NMID):
                nc.vector.tensor_scalar(cjunk[:], g[:], mids[:, i:i + 1], None,
                                        AL.is_gt, op1=AL.add,
                                        accum_out=cnt7[:, i:i + 1])
            cps = psB([128, NMID])
            nc.tensor.matmul(cps[:], cs["EE"][:], cnt7[:], start=True,
                             stop=True)
            flags = pool.tile([128, NMID], f32, tag="s_flag",
                              name=f"flag_{next(_n)}", bufs=2)
            nc.vector.tensor_scalar(flags[:], cps[:], KF, None, AL.is_gt)
            s16 = pool.tile([128, 1], f32, tag="s_s16", name=f"s16_{next(_n)}",
                            bufs=2)
            nc.vector.reduce_sum(s16[:],
                                 flags.rearrange("p (o j) -> p o j", o=1)[:],
                                 axis=AX.X)
            step = pool.tile([128, 1], f32, tag="s_step",
                             name=f"step_{next(_n)}", bufs=2)
            nc.vector.tensor_mul(step[:], s16[:], w8[:])
            lo2 = pool.tile([128, 1], f32, tag="s_lo", name=f"lo_{next(_n)}",
                            bufs=2)
            nc.vector.tensor_add(lo2[:], lo[:], step[:])
            hi2 = pool.tile([128, 1], f32, tag="s_hi", name=f"hi_{next(_n)}",
                            bufs=2)
            nc.vector.tensor_add(hi2[:], lo2[:], w8[:])
            lo, hi = lo2, hi2

        # global masked-out count -> kick collective early (overlaps thr/mask)
        cnt_f = mid("cnt_f", [128, 1])
        nc.vector.tensor_scalar(cjunk[:], g[:], hi[:], None, AL.is_gt,
                                op1=AL.add, accum_out=cnt_f[:])
        totp = psC([1, 1])
        nc.tensor.matmul(totp[:], cs["ones128"][:], cnt_f[:], start=True,
                         stop=True)
        tot_above = sbuf_copy(totp, "tot_above")
        cnt_i = dram.tile([1, 1], f32, tag="cnt_i")
        nc.sync.dma_start(cnt_i[:], tot_above[:])
        cnt_o = dram.tile([1, 1], f32, tag="cnt_o")
        nc.gpsimd.collective_compute(
            "AllReduce", AL.add, replica_groups=[list(range(NCORES))],
            ins=[cnt_i.opt()], outs=[cnt_o.opt()])

        # thr = rowmax(g where g <= hi); hi already replicated per partition
        selm = mid("selm", [128, 256], u8)
        nc.vector.tensor_scalar(selm[:], g[:], hi[:], None, AL.is_le)
        gm = mid("gmz")
        nc.gpsimd.memset(gm[:], -1.0e38)
        nc.vector.copy_predicated(gm[:], selm[:], g[:])
        gmx = mid("gmx", [128, 1])
        nc.vector.tensor_reduce(gmx[:], gm[:], axis=AX.X, op=AL.max)
        tpx = psB([1, 128])
        nc.tensor.transpose(tpx[:], gmx[:], ident[:])
        sx = sbuf_copy(tpx, "sx")
        redx = pool.tile([1, 16], f32, tag="redx")
        nc.vector.tensor_reduce(redx[:],
                                sx.rearrange("o (b q) -> o b q", q=8)[:],
                                axis=AX.X, op=AL.max)
        tpx2 = psB([16, 1])
        nc.tensor.transpose(tpx2[:], redx[:], ident[0:1, 0:1])
        thr16 = sbuf_copy(tpx2, "thr16")
        thrbc = psC([128, 1])
        nc.tensor.matmul(thrbc[:], cs["E16b"][:], thr16[:], start=True,
                         stop=True)
        mask01 = mid("mask01")
        nc.vector.tensor_scalar(mask01[:], g[:], thrbc[:], None, AL.is_le)

        # scale = 262144 / (262144 - total_above)
        tota = pool.tile([1, 1], f32, tag="tota")
        nc.sync.dma_start(tota[:], cnt_o[:])
        scl = pool.tile([1, 1], f32, tag="scl")
        nc.vector.tensor_scalar(scl[:], tota[:], -1.0, 262144.0, AL.mult,
                                op1=AL.add)
        nc.vector.reciprocal(scl[:], scl[:])
        nc.vector.tensor_scalar(scl[:], scl[:], 262144.0, None, AL.mult)
        sclbc = psC([128, 1])
        nc.tensor.matmul(sclbc[:], cs["ones1"][:], scl[:], start=True, stop=True)
        maskS = mid("maskS")
        nc.vector.tensor_scalar(maskS[:], mask01[:], sclbc[:], None, AL.mult)

        # mask columns: smT_h[p, b*8+q] = scaled mask at c' = q*256+h*128+p
        smt_list = []
        for h in range(2):
            tph = psB([128, 128])
            nc.tensor.transpose(tph[:], maskS[:, h * 128:(h + 1) * 128],
                                ident[:])
            sm = pool.tile([128, 128], f32, tag=f"smT{h}")
            nc.scalar.copy(sm[:], tph[:])
            smt_list.append(sm)

        # =========== PHASE C: mask-multiply from cache, store ===========
        ov = out_d.rearrange("b (p jj) h -> b p (jj h)", p=128)
        nhalf = 0
        for b in range(BL):
            for h in range(2):
                ot = xpool.tile([128, HFREE], f32, tag="xa")
                for jh in range(NJ // 2):
                    jj = h * 8 + jh
                    col = b * 8 + jj // 2
                    smcol = smt_list[jj % 2][:, col:col + 1]
                    src = cache[:,
                                b * FREE_B + jj * HW:b * FREE_B + (jj + 1) * HW]
                    dst = ot[:, jh * HW:(jh + 1) * HW]
                    if jj % 4 == 3:
                        nc.scalar.activation(dst, src, AF.Copy, scale=smcol)
                    else:
                        nc.vector.tensor_scalar(dst, src, smcol, None, AL.mult)
                dstv = ov[b, :, h * HFREE:(h + 1) * HFREE]
                if nhalf % 2 == 0:
                    nc.sync.dma_start(dstv, ot[:])
                else:
                    nc.gpsimd.dma_start(dstv, ot[:])
                nhalf += 1

    nc.finalize()
    return nc


def kernel(x, r, ratio, rho):
    x = np.ascontiguousarray(np.asarray(x, dtype=np.float32))
    r = np.ascontiguousarray(np.asarray(r, dtype=np.float32))
    ratio_f = float(np.asarray(ratio))
    rho_f = float(np.asarray(rho))
    k = int(ratio_f * C)
    key = (k, np.float32(rho_f).tobytes())
    if key not in _CACHE:
        _CACHE[key] = build(k, rho_f)
    nc = _CACHE[key]

    consts = _consts()
    xr = x.reshape(B, C, HW)
    in_maps = []
    for c in range(NCORES):
        envm = np.zeros((128, 128), np.float32)
        e = c // 2
        envm[:, e * 32:(e + 1) * 32] = 1.0
        m = {"x": np.ascontiguousarray(xr[c * BL:(c + 1) * BL]),
             "r": np.ascontiguousarray(r[c * BL:(c + 1) * BL]),
             "envm": envm}
        m.update(consts)
        in_maps.append(m)
    res = run_bass_kernel_spmd(nc, in_maps, core_ids=list(range(NCORES)),
                               tmpdir=os.environ.get("BASS_TMPDIR"))
    LAST["res"] = res
    out = np.concatenate([res.results[c]["out"].reshape(BL, C, HW)
                          for c in range(NCORES)], axis=0)
    return out.reshape(B, C, 14, 14)
